# revision 41
# baseline (speedup 1.0000x reference)
"""Trainium2 Bass kernel for BiDACPI (GAT + CNN + bidirectional attention).

Data-parallel over batch b=16 across 8 NeuronCores (2 graphs per core).
Self-contained: hardcodes all shapes; host-side preprocessing only reshapes /
transposes weights and converts index tensors.

v3: attention logits z = mask + src_i + dst_j are built ON THE PE
(fp8-DoubleRow identity x mask matmul + K=2 rank-2 matmul of
[dst;ones]^T [ones;src]), eliminating the DVE broadcast/accumulator ops
that dominated v2. Heads are processed in pairs so softmax-normalize +
elu run on packed [128, 512] tiles. fp16 matmuls; f32 on DVE
scalar_tensor_tensor paths (fp16 is slower there); fp16 only where DVE
tensor_scalar/copy 4x modes apply. Pooling is folded into activation
accum_out (masks are spec'd fill=ones).
"""
import numpy as np

import concourse.bass as bass
import concourse.mybir as mybir
import concourse.tile as tile
from concourse import bacc

F32 = mybir.dt.float32
F16 = mybir.dt.float16
F8 = mybir.dt.float8e5
F8E4 = mybir.dt.float8e4
I32 = mybir.dt.int32
AT = mybir.AluOpType
AF = mybir.ActivationFunctionType
DR = mybir.MatmulPerfMode.DoubleRow

# Problem constants
B = 16
NCORES = 8
G = B // NCORES          # graphs per core
N = 512                  # atoms per graph
L = 1024                 # amino length
CD = 128                 # comp_dim
PD = 128                 # prot_dim
GD = 64                  # gat_dim
H = 4                    # heads
LAT = 128                # latent
NA = 100                 # num_atom
NAM = 30                 # num_amino
LC = 3                   # conv layers
KW = 11                  # conv kernel width
ALPHA = 0.2
MASKNEG = -28672.0       # fp8e5-exact additive mask
NT = N // 128            # 4 j-chunks
PADL = KW // 2
PVW = PADL + L + PADL + 2  # padded pv width (1036, even)

MASK_FP8_DR = True       # fp8e5 identity-mask matmul
LEAKY_SPLIT = 5          # unused
CONV_FP8 = False         # conv band matmuls in fp8e4m3 + DoubleRow i-pairs
NPR = (KW + 1) // 2      # 6 DoubleRow pairs
PV8W = 1040              # fp8 dual-plane width (16B-aligned plane stride)
ZBIG = 128.0             # adjacency scale: z = ZBIG*A01 + src - ZBIG + dst
NZL = 5                  # z lhsT planes: 4 heads + out layer
MIT8O = NZL * 2 * 128    # MiT8 offset in blob8 (zlhs first)
B8W = MIT8O + (LC * NPR * 2 * PD if CONV_FP8 else 0)


def build_core_program(debug=False, mm_bf16=False, dump=False):
    """debug=True builds the CoreSim-compatible variant (no Prelu; no
    activation accum_out)."""
    if debug:
        nc = bacc.Bacc(None, target_bir_lowering=False, debug=True)
    else:
        nc = bacc.Bacc(None)
    MD = F16
    use_prelu = not debug

    # ---- DRAM I/O ----
    d_atoms = nc.dram_tensor("atoms_f", [G, N], MD, kind="ExternalInput")
    d_amask = nc.dram_tensor("atoms_mask", [G, N], F32, kind="ExternalInput")
    # ladj8[g, p, t, i] = binary adjacency A01 for edge j=t*128+p -> i
    d_ladj8 = nc.dram_tensor("ladj8", [G, 128, NT, N], F8E4,
                             kind="ExternalInput")
    d_amino = nc.dram_tensor("amino_f", [G, L], MD, kind="ExternalInput")
    d_pmask = nc.dram_tensor("amino_mask", [G, L], F32, kind="ExternalInput")
    # packed weight blobs (one DMA each): offsets must match preprocess()
    W16O = dict(Eat=0, Eam=128, Wg=256, Wa2h=512, Wgo=708,
                Wgoa2=964, E2=1100, Wc=1228, Wa=1356, MiT=1484)
    W16N = 1484 if CONV_FP8 else 1484 + LC * KW * PD
    WFO = dict(wa1c=0, bc=2, ba=3, pw=4, pb=6)
    WFN = 7
    d_blob16 = nc.dram_tensor("blob16", [128, W16N], MD,
                              kind="ExternalInput")
    d_blobf = nc.dram_tensor("blobf", [128, WFN], F32, kind="ExternalInput")
    d_blob8 = nc.dram_tensor("blob8", [128, B8W], F8E4,
                             kind="ExternalInput")
    d_cb = nc.dram_tensor("conv_b", [LC, 1], F32, kind="ExternalInput")
    d_out = nc.dram_tensor("out", [G, 1], F32, kind="ExternalOutput")
    d_dbg = {}
    if dump:
        for nm, shp in [("U0", [128, NT, N]), ("zm0", [128, NT, N]),
                        ("m01", [128, N]), ("m23", [128, N]),
                        ("xT", [128, N]), ("rinv0", [1, N]),
                        ("pv3", [128, PVW]), ("comp", [LAT, 1]),
                        ("prot", [LAT, 1])]:
            d_dbg[nm] = nc.dram_tensor("dbg_" + nm, shp, F32,
                                       kind="ExternalOutput")

    with tile.TileContext(nc) as tc:
        with (
            tc.tile_pool(name="const", bufs=1) as cpool,
            tc.tile_pool(name="work", bufs=1) as wpool,
            tc.tile_pool(name="att", bufs=1) as apool,
            tc.tile_pool(name="rows", bufs=1) as rpool,
            tc.tile_pool(name="ps_sq", bufs=1, space="PSUM") as pssq,
            tc.tile_pool(name="ps_hp", bufs=2, space="PSUM") as pshp,
            tc.tile_pool(name="ps_z", bufs=2, space="PSUM") as psz,
            tc.tile_pool(name="ps_cv", bufs=1, space="PSUM") as pscv,
        ):
            # queue heads: iota first on gpsimd (gates one-hots), const
            # memsets first on DVE
            ioi = cpool.tile([128, L], I32)
            nc.gpsimd.iota(ioi, pattern=[[0, L]], base=0,
                           channel_multiplier=1)
            ones_col = cpool.tile([128, 1], F32)
            nc.vector.memset(ones_col, 1.0)
            ones16c = cpool.tile([128, 2], MD)
            nc.vector.memset(ones16c, 1.0)
            onesrow = cpool.tile([1, 128], MD)
            nc.vector.memset(onesrow, 1.0)
            iof = cpool.tile([128, L], F32)
            nc.vector.tensor_copy(iof, ioi)

            # ---- input DMAs (phase-1-critical first on each queue) ----
            g_in = []
            rows_in = []
            for g in range(G):
                arow = rpool.tile([1, N], MD, tag="gin1k", bufs=4,
                                  name="arow")
                nc.sync.dma_start(out=arow, in_=d_atoms[g : g + 1, :])
                prow = rpool.tile([1, L], MD, tag="gin2k", bufs=4,
                                  name="prow")
                nc.sync.dma_start(out=prow, in_=d_amino[g : g + 1, :])
                rows_in.append((arow, prow))

            # ---- weights: blob DMAs + cb; masks-first on scalar queue ----
            blob16 = cpool.tile([128, W16N], MD, tag="blob16", name="blob16")
            nc.sync.dma_start(out=blob16[:, :256], in_=d_blob16[:, :256])
            nc.sync.dma_start(out=blob16[:, 256:708], in_=d_blob16[:, 256:708])
            blobf = cpool.tile([128, WFN], F32, tag="blobf", name="blobf")
            nc.sync.dma_start(out=blobf, in_=d_blobf[:, :])
            blob8 = cpool.tile([128, B8W], F8E4, tag="blob8", name="blob8")
            nc.scalar.dma_start(out=blob8[:, :MIT8O], in_=d_blob8[:, :MIT8O])
            cb = cpool.tile([128, LC], F32)
            nc.gpsimd.dma_start(
                out=cb,
                in_=bass.AP(tensor=d_cb, offset=0,
                            ap=[[0, 128], [1, LC], [0, 1]]),
            )
            for g in range(G):
                # dual-plane rhs: [:, 0] = A01 (contiguous DMA), [:, 1, t, :]
                # = per-map plane (avT8 for heads / q8 for the out layer)
                ladj8 = apool.tile([128, 2, NT, N], F8E4, tag=f"ladj{g}",
                                   name="ladjz")
                if g == 0:
                    nc.gpsimd.dma_start(out=ladj8[:, 0], in_=d_ladj8[g])
                else:
                    nc.scalar.dma_start(out=ladj8[:, 0], in_=d_ladj8[g])
                amrow = rpool.tile([1, N], F32, tag="gin2kf", bufs=4,
                                   name="amrow")
                nc.gpsimd.dma_start(out=amrow, in_=d_amask[g : g + 1, :])
                pmrow = rpool.tile([1, L], F32, tag="gin4kf", bufs=4,
                                   name="pmrow")
                nc.gpsimd.dma_start(out=pmrow, in_=d_pmask[g : g + 1, :])
                g_in.append((rows_in[g][0], rows_in[g][1], ladj8, amrow,
                             pmrow))
            nc.scalar.dma_start(out=blob16[:, 708:], in_=d_blob16[:, 708:])
            if CONV_FP8:
                nc.scalar.dma_start(out=blob8[:, MIT8O:],
                                    in_=d_blob8[:, MIT8O:])

            def w16(nm, n, rows=128):
                return blob16[0:rows, W16O[nm] : W16O[nm] + n]

            Eat = w16("Eat", CD)
            Eam = w16("Eam", PD, NAM)
            Wa2h = w16("Wa2h", H)
            Wgoa2 = w16("Wgoa2", 2)
            E2 = w16("E2", 128, 33)
            Wc = w16("Wc", LAT)
            Wa = w16("Wa", LAT)
            Wg_flat = w16("Wg", H * GD)
            wa1c0 = blobf[:, WFO["wa1c"] : WFO["wa1c"] + 1]
            wa1c1 = blobf[:, WFO["wa1c"] + 1 : WFO["wa1c"] + 2]
            bc = blobf[0:LAT, WFO["bc"] : WFO["bc"] + 1]
            ba = blobf[0:LAT, WFO["ba"] : WFO["ba"] + 1]
            pw = blobf[0:LAT, WFO["pw"] : WFO["pw"] + 2]
            pb = blobf[0:1, WFO["pb"] : WFO["pb"] + 1]

            def MiT_v(lyr, i):
                off = W16O["MiT"] + (lyr * KW + i) * PD
                return blob16[:, off : off + PD]

            def Wgo_v(c):
                off = W16O["Wgo"] + c * CD
                return blob16[:, off : off + CD]

            def MiT8_v(lyr, pr):
                off = MIT8O + (lyr * NPR + pr) * 2 * PD
                return bass.AP(tensor=blob8.tensor,
                               offset=blob8.offset + off,
                               ap=[blob8.ap[0], [PD, 2], [1, PD]])

            def zlhs_v(li):
                # [128, 2, 128] DR lhsT: plane0 = ZBIG*I, plane1 = src-maker
                off = li * 2 * 128
                return bass.AP(tensor=blob8.tensor,
                               offset=blob8.offset + off,
                               ap=[blob8.ap[0], [128, 2], [1, 128]])

            def ladjz_rhs(g, t):
                # [128, 2, N] DR rhs: plane0 = A01 chunk t, plane1 = per-map
                lz = g_in[g][2]
                return bass.AP(tensor=lz.tensor,
                               offset=lz.offset + t * N,
                               ap=[lz.ap[0], [NT * N, 2], [1, N]])

            def leaky_act(out, in_, alpha, bias=None, accum_out=None):
                if use_prelu:
                    kw = {}
                    if bias is not None:
                        kw["bias"] = bias
                    if accum_out is not None:
                        kw["accum_out"] = accum_out
                    nc.scalar.activation(out=out, in_=in_, func=AF.Prelu,
                                         alpha=alpha, **kw)
                    return
                src = in_
                if bias is not None:
                    t = wpool.tile(list(out.shape), F32, tag="lk_t", bufs=4,
                                   name="lkb")
                    nc.scalar.activation(out=t, in_=in_, func=AF.Identity,
                                         bias=bias)
                    src = t
                nc.vector.scalar_tensor_tensor(
                    out=out, in0=src, scalar=alpha, in1=src,
                    op0=AT.mult, op1=AT.max)
                if accum_out is not None:
                    scr = wpool.tile(list(out.shape), F32, tag="lk_t2",
                                     bufs=4, name="lks")
                    nc.vector.tensor_scalar(out=scr, in0=out, scalar1=1.0,
                                            scalar2=0.0, op0=AT.mult,
                                            op1=AT.add, accum_out=accum_out)

            def dump_t(nm, src_ap, shape):
                if not dump:
                    return
                t = wpool.tile(shape, F32, tag="dumpbuf", bufs=2,
                               name="dump" + nm)
                nc.vector.tensor_copy(t, src_ap)
                dst = d_dbg[nm]
                sl = tuple([slice(None)] * len(shape))
                nc.sync.dma_start(out=dst[sl], in_=t)

            st = [dict() for _ in range(G)]

            # ================== phase 1: embeddings ==================
            for g in range(G):
                arow, prow, ladj8, amrow, pmrow = g_in[g]
                # atom one-hot -> avT [CD, N] (broadcast via PE)
                ab_ps = pssq.tile([128, N], F32, tag="sq", name="ab_ps")
                nc.tensor.matmul(ab_ps, onesrow, arow, start=True, stop=True)
                ohA = wpool.tile([128, N], MD, tag="t1k", bufs=6, name="ohA")
                nc.vector.tensor_tensor(out=ohA, in0=ab_ps, in1=iof[:, :N],
                                        op=AT.is_equal)
                avT_ps = pssq.tile([128, N], F32, tag="sq", name="avT_ps")
                nc.tensor.matmul(avT_ps, Eat, ohA, start=True, stop=True)
                avT = wpool.tile([128, N], MD, tag=f"avT{g}", bufs=1,
                                 name="avT")
                nc.scalar.copy(avT, avT_ps)
                st[g]["avT"] = avT

                # amino one-hot -> padded pv
                # dual-plane fp8 pv: plane1[k] = plane0[k+1] so DoubleRow
                # i-pairs read k-tiles at a 16B-aligned plane stride
                if CONV_FP8:
                    pv = apool.tile([PD, 2, PV8W], F8E4, tag=f"pv{g}_0",
                                    bufs=1, name="pv")
                    nc.vector.memset(pv[:, 0, :PADL], 0.0)
                    nc.vector.memset(pv[:, 0, PADL + L :], 0.0)
                    nc.vector.memset(pv[:, 1, : PADL - 1], 0.0)
                    nc.vector.memset(pv[:, 1, PADL - 1 + L :], 0.0)
                else:
                    pv = apool.tile([PD, PVW], MD, tag=f"pv{g}_0", bufs=1,
                                    name="pv")
                    nc.vector.memset(pv[:, :PADL], 0.0)
                    nc.vector.memset(pv[:, PADL + L :], 0.0)
                for c in range(2):
                    pb_ps = pssq.tile([128, N], F32, tag="sq", name="pb_ps")
                    nc.tensor.matmul(pb_ps, onesrow,
                                     prow[:, c * 512 : (c + 1) * 512],
                                     start=True, stop=True)
                    ohP = wpool.tile([NAM, N], MD, tag="t1k", bufs=6,
                                     name="ohP")
                    nc.vector.tensor_tensor(
                        out=ohP, in0=pb_ps[:NAM, :],
                        in1=iof[:NAM, c * 512 : (c + 1) * 512],
                        op=AT.is_equal)
                    pvT_ps = pscv.tile([PD, 512], F32, tag="cv",
                                       name="pvT_ps")
                    nc.tensor.matmul(pvT_ps, Eam, ohP, start=True, stop=True)
                    if CONV_FP8:
                        nc.scalar.copy(
                            pv[:, 0, PADL + c * 512 : PADL + (c + 1) * 512],
                            pvT_ps)
                        nc.vector.tensor_copy(
                            pv[:, 1, PADL - 1 + c * 512 :
                               PADL - 1 + (c + 1) * 512],
                            pv[:, 0, PADL + c * 512 : PADL + (c + 1) * 512])
                    else:
                        nc.scalar.copy(
                            pv[:, PADL + c * 512 : PADL + (c + 1) * 512],
                            pvT_ps)
                st[g]["pv"] = pv

            # conv machinery + L1 early (warms the PE during phase 2)
            # ================== conv machinery ==================
            conv_steps = []

            def make_conv_layer(lyr):
                pvo_l, cv = [], {}
                last = lyr == LC - 1
                for g in range(G):
                    if last:
                        pvo = apool.tile([PD, PVW], MD, tag=f"pvf{g}",
                                         bufs=1, name="pvo")
                    elif CONV_FP8:
                        pvo = apool.tile([PD, 2, PV8W], F8E4,
                                         tag=f"pv{g}_{1 - lyr % 2}", bufs=1,
                                         name="pvo")
                    else:
                        pvo = apool.tile([PD, PVW], MD,
                                         tag=f"pv{g}_{1 - lyr % 2}", bufs=1,
                                         name="pvo")
                    pvo_l.append(pvo)
                steps = []

                def mk_group(g, c):
                    def run():
                        if c == 0:
                            if last or not CONV_FP8:
                                nc.vector.memset(pvo_l[g][:, :PADL], 0.0)
                                nc.vector.memset(pvo_l[g][:, PADL + L :],
                                                 0.0)
                            else:
                                nc.vector.memset(pvo_l[g][:, 0, :PADL], 0.0)
                                nc.vector.memset(
                                    pvo_l[g][:, 0, PADL + L :], 0.0)
                                nc.vector.memset(
                                    pvo_l[g][:, 1, : PADL - 1], 0.0)
                                nc.vector.memset(
                                    pvo_l[g][:, 1, PADL - 1 + L :], 0.0)
                        cv[g] = pscv.tile([PD, 512], F32, tag="cv",
                                          name=f"cv{g}")
                        pv = st[g]["pv"]
                        if CONV_FP8:
                            for pr in range(NPR):
                                nc.tensor.matmul(
                                    cv[g], MiT8_v(lyr, pr),
                                    bass.AP(tensor=pv.tensor,
                                            offset=pv.offset + c * 512
                                            + 2 * pr,
                                            ap=[pv.ap[0], [PV8W, 2],
                                                [1, 512]]),
                                    start=(pr == 0), stop=(pr == NPR - 1),
                                    perf_mode=DR)
                        else:
                            for i in range(KW):
                                nc.tensor.matmul(
                                    cv[g], MiT_v(lyr, i),
                                    pv[:, c * 512 + i : c * 512 + i + 512],
                                    start=(i == 0), stop=(i == KW - 1))
                    return run

                def mk_relu(g, c):
                    def run():
                        if last or not CONV_FP8:
                            nc.vector.tensor_scalar(
                                out=pvo_l[g][:, PADL + c * 512 :
                                             PADL + (c + 1) * 512],
                                in0=cv[g], scalar1=cb[:, lyr : lyr + 1],
                                scalar2=0.0, op0=AT.add, op1=AT.max)
                        else:
                            nc.vector.tensor_scalar(
                                out=pvo_l[g][:, 0, PADL + c * 512 :
                                             PADL + (c + 1) * 512],
                                in0=cv[g], scalar1=cb[:, lyr : lyr + 1],
                                scalar2=0.0, op0=AT.add, op1=AT.max)
                            nc.vector.tensor_copy(
                                pvo_l[g][:, 1, PADL - 1 + c * 512 :
                                         PADL - 1 + (c + 1) * 512],
                                pvo_l[g][:, 0, PADL + c * 512 :
                                         PADL + (c + 1) * 512])
                    return run

                for c in range(2):
                    for g in range(G):
                        steps.append(mk_group(g, c))
                        steps.append(mk_relu(g, c))

                def finish():
                    for g in range(G):
                        st[g]["pv"] = pvo_l[g]
                    if dump and lyr == LC - 1:
                        dump_t("pv3", pvo_l[0], [128, PVW])
                steps.append(finish)
                return steps

            for lyr in range(LC):
                conv_steps.extend(make_conv_layer(lyr))

            # prot head rides the conv queue: each chunk is ready as soon
            # as the final conv relu for it has run
            opacc = {}

            def mk_prot(g, c):
                def run():
                    pv = st[g]["pv"]
                    pvt = psz.tile([128, 2, N], F32, tag="z", name="pv_ps")
                    pv_ps = pvt[:LAT, 0, :]
                    nc.tensor.matmul(pv_ps, Wa,
                                     pv[:, PADL + c * 512 :
                                        PADL + (c + 1) * 512],
                                     start=True, stop=True)
                    p_v = wpool.tile([LAT, 512], MD, tag="p_v", bufs=4,
                                     name="p_v")
                    pacc = rpool.tile([LAT, 1], F32, tag="c1", bufs=8,
                                      name="pacc")
                    leaky_act(p_v, pv_ps, ALPHA, bias=ba, accum_out=pacc)
                    opacc.setdefault(g, []).append(pacc)
                return run

            for c in range(2):
                for g in range(G):
                    conv_steps.append(mk_prot(g, c))

            def pop_conv(k):
                for _ in range(k):
                    if conv_steps:
                        conv_steps.pop(0)()


            # ================== phase 2: per-graph prep ==================
            for g in range(G):
                avT = st[g]["avT"]
                # all heads' Wh -> whsb_all[p, t, h, 0:64]; col 64 = 1.0
                whsb_all = wpool.tile([128, NT, H, GD + 2], MD,
                                      tag=f"whsb{g}", bufs=1, name="whsb_all")
                nc.vector.memset(whsb_all[:, :, :, GD : GD + 1], 1.0)
                for half in range(2):
                    wh_all = pssq.tile([128, 2, H * GD], F32, tag="sq",
                                       name="wh_all")
                    for t2 in range(2):
                        t = half * 2 + t2
                        nc.tensor.matmul(
                            wh_all[:, t2, :],
                            avT[:, t * 128 : (t + 1) * 128],
                            Wg_flat, start=True, stop=True)
                    nc.scalar.copy(
                        out=bass.AP(
                            tensor=whsb_all.tensor,
                            offset=whsb_all.offset
                            + half * 2 * H * (GD + 2),
                            ap=[whsb_all.ap[0], [H * (GD + 2), 2],
                                [GD + 2, H], [1, GD]]),
                        in_=wh_all)
                st[g]["whsb"] = whsb_all

                # dst bias columns for all heads: dstT[j, t*H+h] =
                # a2_h.Wh_j - ZBIG (the -ZBIG cancels plane0's ZBIG*A01)
                dst_ps = pssq.tile([128, NT * H], F32, tag="sq",
                                   name="dst_ps")
                for t in range(NT):
                    nc.tensor.matmul(dst_ps[:, t * H : (t + 1) * H],
                                     avT[:, t * 128 : (t + 1) * 128], Wa2h,
                                     start=True, stop=True)
                dstT = rpool.tile([128, NT * H], F32, tag=f"dstT{g}", bufs=1,
                                  name="dstT")
                nc.vector.tensor_scalar(out=dstT, in0=dst_ps, scalar1=-ZBIG,
                                        scalar2=None, op0=AT.add)
                st[g]["dstT"] = dstT
                # z rhs plane1 for head maps: avT in fp8e4
                ladjz = g_in[g][2]
                for t in range(NT):
                    nc.vector.tensor_copy(ladjz[:, 1, t, :], avT)


            # ============ attention z/U (per head or out-layer) ============
            def att_zU(g, li, bias4, dump_zu=False):
                """One DR matmul per chunk: z = ZBIG*A01 + src; dst - ZBIG
                enters as the Prelu bias. Returns U [128, NT, N] fp16."""
                ee = apool.tile([128, NT, N], MD, tag="ee", bufs=2, name="ee")
                for th in range(2):
                    zps = psz.tile([128, 2, N], F32, tag="z", name="zps")
                    for t2 in range(2):
                        t = th * 2 + t2
                        nc.tensor.matmul(zps[:, t2, :], zlhs_v(li),
                                         ladjz_rhs(g, t), start=True,
                                         stop=True, perf_mode=DR)
                    for t2 in range(2):
                        t = th * 2 + t2
                        if use_prelu:
                            nc.scalar.activation(out=ee[:, t, :],
                                                 in_=zps[:, t2, :],
                                                 func=AF.Prelu, alpha=ALPHA,
                                                 bias=bias4[t])
                        else:
                            eesc = wpool.tile([128, N], F32, tag="eesc",
                                              bufs=4, name="eesc")
                            nc.scalar.activation(out=eesc, in_=zps[:, t2, :],
                                                 func=AF.Identity,
                                                 bias=bias4[t])
                            nc.vector.scalar_tensor_tensor(
                                out=ee[:, t, :], in0=eesc, scalar=ALPHA,
                                in1=eesc, op0=AT.mult, op1=AT.max)
                if dump_zu:
                    dump_t("zm0", ee, [128, NT, N])
                U = apool.tile([128, NT, N], MD, tag="U", bufs=2, name="U")
                nc.scalar.activation(out=U, in_=ee, func=AF.Exp)
                if dump_zu:
                    dump_t("U0", U, [128, NT, N])
                return U

            def elu_norm_pair(hp0, hp1, dst):
                """Normalize two heads' hp [65, N] (row 64 = rowsum) and
                elu into packed dst [128, N]."""
                rsum2 = rpool.tile([33, N], F32, tag="zrw2", bufs=3,
                                   name="rsum2")
                nc.gpsimd.memset(rsum2, 1.0)
                nc.vector.tensor_copy(rsum2[0:1, :], hp0[64:65, :])
                nc.vector.tensor_copy(rsum2[32:33, :], hp1[64:65, :])
                rinv2 = rpool.tile([33, N], F32, tag="zrw2", bufs=3,
                                   name="rinv2")
                nc.vector.reciprocal_approx_fast(out=rinv2, in_=rsum2)
                rinv16 = rpool.tile([33, N], MD, tag="ri16", bufs=2,
                                    name="rinv16")
                nc.vector.tensor_scalar(out=rinv16, in0=rinv2, scalar1=1.0,
                                        scalar2=None, op0=AT.mult)
                rb_ps = pssq.tile([128, N], F32, tag="sq", name="rb_ps")
                nc.tensor.matmul(rb_ps, E2, rinv16, start=True, stop=True)
                rb = wpool.tile([128, N], MD, tag="rb", bufs=2, name="rb")
                nc.vector.tensor_copy(rb, rb_ps)
                hpn = wpool.tile([128, N], MD, tag="elu", bufs=4, name="hpn")
                nc.vector.scalar_tensor_tensor(
                    out=hpn[0:64, :], in0=hp0[:64, :], scalar=1.0,
                    in1=rb[0:64, :], op0=AT.mult, op1=AT.mult)
                nc.vector.scalar_tensor_tensor(
                    out=hpn[64:128, :], in0=hp1[:64, :], scalar=1.0,
                    in1=rb[64:128, :], op0=AT.mult, op1=AT.mult)
                xm = wpool.tile([128, N], MD, tag="elu", bufs=4, name="xm")
                nc.vector.tensor_scalar(out=xm, in0=hpn, scalar1=0.0,
                                        scalar2=None, op0=AT.min)
                em = wpool.tile([128, N], MD, tag="elu", bufs=4, name="em")
                nc.scalar.activation(out=em, in_=xm, func=AF.Exp)
                nc.vector.scalar_tensor_tensor(
                    out=dst, in0=em, scalar=-1.0, in1=hpn,
                    op0=AT.add, op1=AT.max)

            # ================== phase 3: head maps + conv ==================
            for g in range(G):
                m01 = wpool.tile([128, N], MD, tag=f"m01{g}", bufs=1,
                                 name="m01")
                m23 = wpool.tile([128, N], MD, tag=f"m23{g}", bufs=1,
                                 name="m23")
                st[g]["multi"] = (m01, m23)
                st[g]["hp"] = {}

            # software-pipelined attention: emit z/U of map k+1 before the
            # hp matmuls of map k so the PE never stalls on the exp chain
            pend = None          # (g, h, U) waiting for its hp emission
            pair_hps = {}

            def emit_hp(g, h, U):
                whsb_all = st[g]["whsb"]
                hp_t = pshp.tile([128, N], F32, tag="hp", name="hp_ps")
                hp = hp_t[0:65, :]
                for t in range(NT):
                    nc.tensor.matmul(hp, whsb_all[:, t, h, : GD + 1],
                                     U[:, t, :],
                                     start=(t == 0), stop=(t == NT - 1))
                pair_hps.setdefault(g, []).append(hp)
                if len(pair_hps[g]) == 2:
                    hps = pair_hps.pop(g)
                    elu_norm_pair(hps[0], hps[1], st[g]["multi"][h // 2])

            # --- per-graph tail chunks (pipelined into later head maps) ---
            oU, ohp, orb = {}, {}, {}

            def tail_a1(g):
                m01, m23 = st[g]["multi"]
                multi = [m01, m23]
                wh2_ps = pssq.tile([128, NT, CD], F32, tag="sq",
                                   name="wh2_ps")
                for t in range(NT):
                    for c in range(2):
                        nc.tensor.matmul(
                            wh2_ps[:, t, :],
                            multi[c][:, t * 128 : (t + 1) * 128],
                            Wgo_v(c), start=(c == 0), stop=(c == 1))
                wh2sb = wpool.tile([128, NT, CD], MD, tag=f"wh2{g}", bufs=1,
                                   name="wh2sb")
                for t in range(NT):
                    nc.vector.tensor_copy(wh2sb[:, t, :], wh2_ps[:, t, :])
                st[g]["wh2sb"] = wh2sb
                # out-layer z rhs plane1: q[k,i] = a1go[k,0]*m01 + a1go[k,1]
                # *m23 (ones-lhsT row-sums it into src2)
                qt1 = wpool.tile([128, N], MD, tag="qt", bufs=4, name="qt1")
                nc.vector.tensor_scalar(out=qt1, in0=m01, scalar1=wa1c0,
                                        scalar2=None, op0=AT.mult)
                qt2 = wpool.tile([128, N], MD, tag="qt", bufs=4, name="qt2")
                nc.vector.tensor_scalar(out=qt2, in0=m23, scalar1=wa1c1,
                                        scalar2=None, op0=AT.mult)
                ladjz = g_in[g][2]
                for t in range(NT):
                    nc.vector.tensor_tensor(out=ladjz[:, 1, t, :], in0=qt1,
                                            in1=qt2, op=AT.add)
                # out-layer dst bias columns: dstT2[j, t] = a2go.xo_j - ZBIG
                d2_ps = pssq.tile([128, NT], F32, tag="sq", name="d2_ps")
                for t in range(NT):
                    for c in range(2):
                        nc.tensor.matmul(
                            d2_ps[:, t : t + 1],
                            multi[c][:, t * 128 : (t + 1) * 128],
                            Wgoa2[:, c : c + 1],
                            start=(c == 0), stop=(c == 1))
                dstT2 = rpool.tile([128, NT], F32, tag=f"d2T{g}", bufs=1,
                                   name="dstT2")
                nc.vector.tensor_scalar(out=dstT2, in0=d2_ps, scalar1=-ZBIG,
                                        scalar2=None, op0=AT.add)
                st[g]["dstT2"] = dstT2

            def tail_a2(g):
                dstT2 = st[g]["dstT2"]
                oU[g] = att_zU(g, H,
                               [dstT2[:, t : t + 1] for t in range(NT)])

            def tail_a3(g):
                wh2sb = st[g]["wh2sb"]
                hp2 = pshp.tile([128, N], F32, tag="hp", name="hp2_ps")
                for t in range(NT):
                    nc.tensor.matmul(hp2, wh2sb[:, t, :], oU[g][:, t, :],
                                     start=(t == 0), stop=(t == NT - 1))
                rs_t = psz.tile([128, 2, N], F32, tag="z", name="rs_ps")
                rs_ps = rs_t[:, 0, :]
                for t in range(NT):
                    nc.tensor.matmul(rs_ps[0:1, :], ones16c[:, 0:1],
                                     oU[g][:, t, :],
                                     start=(t == 0), stop=(t == NT - 1))
                rinv = rpool.tile([1, N], F32, tag="zrwo", bufs=4,
                                  name="rinvo")
                nc.vector.reciprocal_approx_fast(out=rinv, in_=rs_ps[0:1, :])
                rinv16 = rpool.tile([1, N], MD, tag="ri16o", bufs=2,
                                    name="rinv16o")
                nc.vector.tensor_scalar(out=rinv16, in0=rinv, scalar1=1.0,
                                        scalar2=None, op0=AT.mult)
                rb_ps = pssq.tile([128, N], F32, tag="sq", name="rb2_ps")
                nc.tensor.matmul(rb_ps, onesrow, rinv16, start=True,
                                 stop=True)
                rb = wpool.tile([128, N], MD, tag="rb", bufs=2, name="rbo")
                nc.vector.tensor_copy(rb, rb_ps)
                ohp[g] = hp2
                orb[g] = rb

            def tail_b(g):
                # xT = elu(hp2 / rowsum), then comp head + pooling
                xT = wpool.tile([CD, N], MD, tag=f"xT{g}", bufs=1, name="xT")
                hpn = wpool.tile([128, N], MD, tag="elu", bufs=4,
                                 name="hpno")
                nc.vector.scalar_tensor_tensor(
                    out=hpn, in0=ohp[g], scalar=1.0, in1=orb[g],
                    op0=AT.mult, op1=AT.mult)
                xm = wpool.tile([128, N], MD, tag="elu", bufs=4, name="xmo")
                nc.vector.tensor_scalar(out=xm, in0=hpn, scalar1=0.0,
                                        scalar2=None, op0=AT.min)
                em = wpool.tile([128, N], MD, tag="elu", bufs=4, name="emo")
                nc.scalar.activation(out=em, in_=xm, func=AF.Exp)
                nc.vector.scalar_tensor_tensor(
                    out=xT, in0=em, scalar=-1.0, in1=hpn,
                    op0=AT.add, op1=AT.max)
                av_t = pshp.tile([128, N], F32, tag="hp", name="av_ps")
                av = av_t[:LAT, :]
                nc.tensor.matmul(av, Wc, xT, start=True, stop=True)
                avec = wpool.tile([LAT, N], MD, tag="avec", bufs=2,
                                  name="avec")
                leaky_act(avec, av, ALPHA, bias=bc)
                av2_t = pshp.tile([128, N], F32, tag="hp", name="av2_ps")
                av2 = av2_t[:LAT, :]
                nc.tensor.matmul(av2, Wa, avec, start=True, stop=True)
                a_v = wpool.tile([LAT, N], MD, tag="avec", bufs=2,
                                 name="a_v")
                comp_acc = rpool.tile([LAT, 1], F32, tag="c1", bufs=8,
                                      name="comp_acc")
                leaky_act(a_v, av2, ALPHA, bias=ba, accum_out=comp_acc)
                amrow = g_in[g][3]
                amscr = rpool.tile([1, N], F32, tag="r2k", bufs=6,
                                   name="amscr")
                amsum = rpool.tile([1, 1], F32, tag="c2", bufs=12,
                                   name="amsum")
                nc.vector.tensor_scalar(out=amscr, in0=amrow, scalar1=1.0,
                                        scalar2=0.0, op0=AT.mult, op1=AT.add,
                                        accum_out=amsum)
                amsb = rpool.tile([128, 1], F32, tag="c2", bufs=12,
                                  name="amsb")
                nc.gpsimd.partition_broadcast(amsb, amsum)
                amr = rpool.tile([128, 1], F32, tag="c2", bufs=12,
                                 name="amr")
                nc.vector.reciprocal(amr, amsb)
                cp = rpool.tile([128, 2], F32, tag="cp", bufs=4, name="cp")
                nc.vector.tensor_scalar(out=cp[:, 0:1], in0=comp_acc,
                                        scalar1=amr, scalar2=None,
                                        op0=AT.mult)
                st[g]["cp"] = cp

            def tail_c(g):
                # prot pooling + prediction (opacc filled via the conv queue)
                cp = st[g]["cp"]
                prot_acc = rpool.tile([LAT, 1], F32, tag="c1", bufs=8,
                                      name="prot_acc")
                nc.vector.tensor_tensor(out=prot_acc, in0=opacc[g][0],
                                        in1=opacc[g][1], op=AT.add)
                pmrow = g_in[g][4]
                pmscr = rpool.tile([1, L], F32, tag="r4k", bufs=2,
                                   name="pmscr")
                pmsum = rpool.tile([1, 1], F32, tag="c2", bufs=12,
                                   name="pmsum")
                nc.vector.tensor_scalar(out=pmscr, in0=pmrow, scalar1=1.0,
                                        scalar2=0.0, op0=AT.mult, op1=AT.add,
                                        accum_out=pmsum)
                pmsb = rpool.tile([128, 1], F32, tag="c2", bufs=12,
                                  name="pmsb")
                nc.gpsimd.partition_broadcast(pmsb, pmsum)
                pmr = rpool.tile([128, 1], F32, tag="c2", bufs=12,
                                 name="pmr")
                nc.vector.reciprocal(pmr, pmsb)
                nc.vector.tensor_scalar(out=cp[:, 1:2], in0=prot_acc,
                                        scalar1=pmr, scalar2=None,
                                        op0=AT.mult)
                lr2 = rpool.tile([128, 2], F32, tag="cp", bufs=4, name="lr2")
                leaky_act(lr2, cp, ALPHA * ALPHA)
                dscr = rpool.tile([128, 2], F32, tag="cp", bufs=4,
                                  name="dscr")
                dacc = rpool.tile([128, 1], F32, tag="c1", bufs=8,
                                  name="dacc")
                nc.vector.scalar_tensor_tensor(
                    out=dscr, in0=lr2, scalar=1.0, in1=pw,
                    op0=AT.mult, op1=AT.mult, accum_out=dacc)
                fin_ps = pssq.tile([128, N], F32, tag="sq", name="fin_ps")
                nc.tensor.matmul(fin_ps[0:1, 0:1], dacc, ones_col,
                                 start=True, stop=True)
                res = rpool.tile([1, 1], F32, tag="c2", bufs=12, name="res")
                nc.scalar.activation(out=res, in_=fin_ps[0:1, 0:1],
                                     func=AF.Identity, bias=pb)
                nc.sync.dma_start(out=d_out[g : g + 1, :], in_=res)

            # schedule: g-major maps; g0's tail chunks interleave g1's maps
            tail_after = {(1, 0): [lambda: tail_a1(0)],
                          (1, 1): [lambda: tail_a2(0)],
                          (1, 2): [lambda: tail_a3(0)],
                          (1, 3): [lambda: tail_b(0)]}
            pop_conv(9)
            for g in range(G):
                dstT = st[g]["dstT"]
                for h in range(H):
                    U = att_zU(g, h,
                               [dstT[:, t * H + h : t * H + h + 1]
                                for t in range(NT)],
                               dump_zu=(dump and g == 0 and h == 0))
                    if pend is not None:
                        emit_hp(*pend)
                    pend = (g, h, U)
                    pop_conv(2)
                    for fn in tail_after.get((g, h), []):
                        fn()
            if pend is not None:
                emit_hp(*pend)
                pend = None
            pop_conv(len(conv_steps))
            tail_a1(1)
            tail_c(0)
            tail_a2(1)
            tail_a3(1)
            tail_b(1)
            tail_c(1)

    return nc


def preprocess(inputs, mm_bf16=False):
    """Host-side prep: shard over cores, transpose/reshape weights."""
    import ml_dtypes
    md = np.float16
    f8 = ml_dtypes.float8_e4m3fn
    atoms = np.asarray(inputs["atoms"]).astype(np.float32)
    atoms_mask = np.asarray(inputs["atoms_mask"]).astype(np.float32)
    adjacency = np.asarray(inputs["adjacency"])
    amino = np.asarray(inputs["amino"]).astype(np.float32)
    amino_mask = np.asarray(inputs["amino_mask"]).astype(np.float32)
    E_atom = np.asarray(inputs["E_atom"]).astype(np.float32)
    E_amino = np.asarray(inputs["E_amino"]).astype(np.float32)
    W_gat = np.asarray(inputs["W_gat"]).astype(np.float32)
    a_gat = np.asarray(inputs["a_gat"]).astype(np.float32)
    W_go = np.asarray(inputs["W_go"]).astype(np.float32)
    a_go = np.asarray(inputs["a_go"]).astype(np.float32)
    W_comp_w = np.asarray(inputs["W_comp_w"]).astype(np.float32)
    W_comp_b = np.asarray(inputs["W_comp_b"]).astype(np.float32)
    conv_w = np.asarray(inputs["conv_w"]).astype(np.float32)
    conv_b = np.asarray(inputs["conv_b"]).astype(np.float32)
    W_att_w = np.asarray(inputs["W_att_w"]).astype(np.float32)
    W_att_b = np.asarray(inputs["W_att_b"]).astype(np.float32)
    pred_w = np.asarray(inputs["pred_w"]).astype(np.float32)
    pred_b = np.asarray(inputs["pred_b"]).astype(np.float32)

    ladjT = (adjacency.transpose(0, 2, 1) > 0).astype(np.float32)
    ladjT_r = np.ascontiguousarray(
        ladjT.reshape(B, NT, 128, N).transpose(0, 2, 1, 3))

    E_atom_pad = np.zeros((128, CD), np.float32)
    E_atom_pad[:NA] = E_atom

    MiT = np.zeros((LC, KW, PD, PD), np.float32)
    din = np.arange(PD)[:, None]
    dout = np.arange(PD)[None, :]
    v = din - dout + (KW // 2)
    valid = (v >= 0) & (v < KW)
    vc = np.clip(v, 0, KW - 1)
    for lyr in range(LC):
        for i in range(KW):
            MiT[lyr, i] = np.where(valid, conv_w[lyr, 0, 0, i, vc], 0.0)
    MiT_r = np.ascontiguousarray(MiT.transpose(2, 0, 1, 3))
    NPRl = (KW + 1) // 2
    MiT8 = np.zeros((LC, NPRl, 2, PD, PD), np.float32)
    for lyr in range(LC):
        for pr in range(NPRl):
            MiT8[lyr, pr, 0] = MiT[lyr, 2 * pr]
            if 2 * pr + 1 < KW:
                MiT8[lyr, pr, 1] = MiT[lyr, 2 * pr + 1]
    MiT8_r = np.ascontiguousarray(MiT8.transpose(3, 0, 1, 2, 4))

    W_gat_r = np.ascontiguousarray(W_gat.transpose(1, 0, 2))
    Wa1h = np.einsum("hpq,hq->ph", W_gat, a_gat[:, :GD])
    Wa2h = np.einsum("hpq,hq->ph", W_gat, a_gat[:, GD:])

    W_go_r = np.ascontiguousarray(
        W_go.reshape(2, 128, CD).transpose(1, 0, 2))
    Wgo_a2 = (W_go @ a_go[CD:]).reshape(2, 128).T     # (128, 2)
    Wgo_a1 = (W_go @ a_go[:CD]).reshape(2, 128).T

    E2 = np.zeros((33, 128), np.float32)
    E2[0, 0:64] = 1.0
    E2[32, 64:128] = 1.0

    # pack fp16 weight blob (offsets mirror kernel W16O)
    W16N = 1484 if CONV_FP8 else 1484 + LC * KW * PD
    blob16 = np.zeros((128, W16N), np.float32)

    def put16(off, arr):
        a = np.asarray(arr, np.float32)
        blob16[: a.shape[0], off : off + a.shape[1]] = a

    put16(0, E_atom_pad)
    put16(128, E_amino)
    put16(256, W_gat_r.reshape(CD, H * GD))
    put16(512, Wa2h)
    put16(708, W_go_r.reshape(128, 2 * CD))
    put16(964, Wgo_a2)
    put16(1100, E2)
    put16(1228, W_comp_w.T)
    put16(1356, W_att_w.T)
    if not CONV_FP8:
        put16(1484, MiT_r.reshape(PD, LC * KW * PD))

    blobf = np.zeros((128, 7), np.float32)
    blobf[:, 0:2] = Wgo_a1
    blobf[:LAT, 2] = W_comp_b
    blobf[:LAT, 3] = W_att_b
    blobf[:LAT, 4] = pred_w[0, :LAT]
    blobf[:LAT, 5] = pred_w[0, LAT:]
    blobf[0, 6] = pred_b[0]

    # fp8 blob: z lhsT planes (ZBIG*I | src-maker) per map, then MiT8
    B8Wl = NZL * 2 * 128 + (LC * NPRl * 2 * PD if CONV_FP8 else 0)
    blob8 = np.zeros((128, B8Wl), np.float32)
    zlarr = np.zeros((128, NZL, 2, 128), np.float32)
    for li in range(NZL):
        zlarr[:, li, 0, :] = ZBIG * np.eye(128, dtype=np.float32)
    for h in range(H):
        zlarr[:, h, 1, :] = Wa1h[:, h : h + 1]
    zlarr[:, H, 1, :] = 1.0
    blob8[:, : NZL * 2 * 128] = zlarr.reshape(128, NZL * 2 * 128)
    if CONV_FP8:
        blob8[:, NZL * 2 * 128 :] = MiT8_r.reshape(PD, LC * NPRl * 2 * PD)

    shared = {
        "blob16": blob16.astype(md),
        "blobf": blobf,
        "blob8": blob8.astype(ml_dtypes.float8_e4m3fn),
        "conv_b": np.ascontiguousarray(conv_b.reshape(LC, 1)),
    }
    in_maps = []
    for c in range(NCORES):
        sl = slice(c * G, (c + 1) * G)
        m = dict(shared)
        m["atoms_f"] = np.ascontiguousarray(atoms[sl]).astype(md)
        m["atoms_mask"] = np.ascontiguousarray(atoms_mask[sl])
        m["ladj8"] = np.ascontiguousarray(ladjT_r[sl]).astype(f8)
        m["amino_f"] = np.ascontiguousarray(amino[sl]).astype(md)
        m["amino_mask"] = np.ascontiguousarray(amino_mask[sl])
        in_maps.append(m)
    return in_maps


_CACHED_NC = None


def kernel(**inputs) -> np.ndarray:
    global _CACHED_NC
    from concourse.bass_utils import run_bass_kernel_spmd

    if _CACHED_NC is None:
        nc = build_core_program()
        nc.finalize()
        _CACHED_NC = nc
    nc = _CACHED_NC
    in_maps = preprocess(inputs)
    res = run_bass_kernel_spmd(nc, in_maps, core_ids=list(range(NCORES)))
    out = np.concatenate([res.results[c]["out"] for c in range(NCORES)], axis=0)
    return out.astype(np.float32)



# revision 53
# speedup vs baseline: 1.0169x; 1.0169x over previous
"""Trainium2 Bass kernel for BiDACPI (GAT + CNN + bidirectional attention).

Data-parallel over batch b=16 across 8 NeuronCores (2 graphs per core).
Self-contained: hardcodes all shapes; host-side preprocessing only reshapes /
transposes weights and converts index tensors.

v3: attention logits z = mask + src_i + dst_j are built ON THE PE
(fp8-DoubleRow identity x mask matmul + K=2 rank-2 matmul of
[dst;ones]^T [ones;src]), eliminating the DVE broadcast/accumulator ops
that dominated v2. Heads are processed in pairs so softmax-normalize +
elu run on packed [128, 512] tiles. fp16 matmuls; f32 on DVE
scalar_tensor_tensor paths (fp16 is slower there); fp16 only where DVE
tensor_scalar/copy 4x modes apply. Pooling is folded into activation
accum_out (masks are spec'd fill=ones).
"""
import numpy as np

import concourse.bass as bass
import concourse.mybir as mybir
import concourse.tile as tile
from concourse import bacc

F32 = mybir.dt.float32
F16 = mybir.dt.float16
F8 = mybir.dt.float8e5
F8E4 = mybir.dt.float8e4
I32 = mybir.dt.int32
AT = mybir.AluOpType
AF = mybir.ActivationFunctionType
DR = mybir.MatmulPerfMode.DoubleRow

# Problem constants
B = 16
NCORES = 8
G = B // NCORES          # graphs per core
N = 512                  # atoms per graph
L = 1024                 # amino length
CD = 128                 # comp_dim
PD = 128                 # prot_dim
GD = 64                  # gat_dim
H = 4                    # heads
LAT = 128                # latent
NA = 100                 # num_atom
NAM = 30                 # num_amino
LC = 3                   # conv layers
KW = 11                  # conv kernel width
ALPHA = 0.2
MASKNEG = -28672.0       # fp8e5-exact additive mask
NT = N // 128            # 4 j-chunks
PADL = KW // 2
PVW = PADL + L + PADL + 2  # padded pv width (1036, even)

MASK_FP8_DR = True       # fp8e5 identity-mask matmul
LEAKY_SPLIT = 5          # unused
CONV_FP8 = False         # conv band matmuls in fp8e4m3 + DoubleRow i-pairs
NPR = (KW + 1) // 2      # 6 DoubleRow pairs
PV8W = 1040              # fp8 dual-plane width (16B-aligned plane stride)
ZBIG = 128.0             # adjacency scale: z = ZBIG*A01 + src - ZBIG + dst
NZL = 5                  # z lhsT planes: 4 heads + out layer
MIT8O = NZL * 2 * 128    # MiT8 offset in blob8 (zlhs first)
B8W = MIT8O + (LC * NPR * 2 * PD if CONV_FP8 else 0)


def build_core_program(debug=False, mm_bf16=False, dump=False):
    """debug=True builds the CoreSim-compatible variant (no Prelu; no
    activation accum_out)."""
    if debug:
        nc = bacc.Bacc(None, target_bir_lowering=False, debug=True)
    else:
        nc = bacc.Bacc(None)
    MD = F16
    use_prelu = not debug

    # ---- DRAM I/O ----
    d_atoms = nc.dram_tensor("atoms_f", [G, N], MD, kind="ExternalInput")
    d_amask = nc.dram_tensor("atoms_mask", [G, N], F32, kind="ExternalInput")
    # ladj8[g, p, t, i] = binary adjacency A01 for edge j=t*128+p -> i
    d_ladj8 = nc.dram_tensor("ladj8", [G, 128, NT, N], F8E4,
                             kind="ExternalInput")
    d_amino = nc.dram_tensor("amino_f", [G, L], MD, kind="ExternalInput")
    d_pmask = nc.dram_tensor("amino_mask", [G, L], F32, kind="ExternalInput")
    # packed weight blobs (one DMA each): offsets must match preprocess()
    W16O = dict(Eat=0, Eam=128, Wg=256, Wa2h=512, Wgo=708,
                Wgoa2=964, E2=1100, Wc=1228, Wa=1356, MiT=1484)
    W16N = 1484 if CONV_FP8 else 1484 + LC * KW * PD
    WFO = dict(wa1c=0, bc=2, ba=3, pw=4, pb=6)
    WFN = 7
    d_blob16 = nc.dram_tensor("blob16", [128, W16N], MD,
                              kind="ExternalInput")
    d_blobf = nc.dram_tensor("blobf", [128, WFN], F32, kind="ExternalInput")
    d_blob8 = nc.dram_tensor("blob8", [128, B8W], F8E4,
                             kind="ExternalInput")
    d_cb = nc.dram_tensor("conv_b", [LC, 1], F32, kind="ExternalInput")
    d_out = nc.dram_tensor("out", [G, 1], F32, kind="ExternalOutput")
    d_dbg = {}
    if dump:
        for nm, shp in [("U0", [128, NT, N]), ("zm0", [128, NT, N]),
                        ("m01", [128, N]), ("m23", [128, N]),
                        ("xT", [128, N]), ("rinv0", [1, N]),
                        ("pv3", [128, PVW]), ("comp", [LAT, 1]),
                        ("prot", [LAT, 1])]:
            d_dbg[nm] = nc.dram_tensor("dbg_" + nm, shp, F32,
                                       kind="ExternalOutput")

    with tile.TileContext(nc) as tc:
        with (
            tc.tile_pool(name="const", bufs=1) as cpool,
            tc.tile_pool(name="work", bufs=1) as wpool,
            tc.tile_pool(name="att", bufs=1) as apool,
            tc.tile_pool(name="rows", bufs=1) as rpool,
            tc.tile_pool(name="ps_sq", bufs=1, space="PSUM") as pssq,
            tc.tile_pool(name="ps_hp", bufs=2, space="PSUM") as pshp,
            tc.tile_pool(name="ps_z", bufs=2, space="PSUM") as psz,
            tc.tile_pool(name="ps_cv", bufs=1, space="PSUM") as pscv,
        ):
            # queue heads: iota first on gpsimd (gates one-hots), const
            # memsets first on DVE
            ioi = cpool.tile([128, L], I32)
            nc.gpsimd.iota(ioi, pattern=[[0, L]], base=0,
                           channel_multiplier=1)
            ones_col = cpool.tile([128, 1], F32)
            nc.vector.memset(ones_col, 1.0)
            ones16c = cpool.tile([128, 2], MD)
            nc.vector.memset(ones16c, 1.0)
            onesrow = cpool.tile([1, 128], MD)
            nc.vector.memset(onesrow, 1.0)
            iof = cpool.tile([128, L], F32)
            nc.vector.tensor_copy(iof, ioi)

            # ---- input DMAs (phase-1-critical first on each queue) ----
            g_in = []
            rows_in = []
            for g in range(G):
                arow = rpool.tile([1, N], MD, tag="gin1k", bufs=4,
                                  name="arow")
                nc.sync.dma_start(out=arow, in_=d_atoms[g : g + 1, :])
                prow = rpool.tile([1, L], MD, tag="gin2k", bufs=4,
                                  name="prow")
                nc.sync.dma_start(out=prow, in_=d_amino[g : g + 1, :])
                rows_in.append((arow, prow))

            # ---- weights: blob DMAs + cb; masks-first on scalar queue ----
            blob16 = cpool.tile([128, W16N], MD, tag="blob16", name="blob16")
            nc.sync.dma_start(out=blob16[:, :256], in_=d_blob16[:, :256])
            nc.sync.dma_start(out=blob16[:, 256:708], in_=d_blob16[:, 256:708])
            blobf = cpool.tile([128, WFN], F32, tag="blobf", name="blobf")
            nc.sync.dma_start(out=blobf, in_=d_blobf[:, :])
            blob8 = cpool.tile([128, B8W], F8E4, tag="blob8", name="blob8")
            nc.scalar.dma_start(out=blob8[:, :MIT8O], in_=d_blob8[:, :MIT8O])
            cb = cpool.tile([128, LC], F32)
            nc.gpsimd.dma_start(
                out=cb,
                in_=bass.AP(tensor=d_cb, offset=0,
                            ap=[[0, 128], [1, LC], [0, 1]]),
            )
            for g in range(G):
                # dual-plane rhs: [:, 0] = A01 (contiguous DMA), [:, 1, t, :]
                # = per-map plane (avT8 for heads / q8 for the out layer).
                # g1's mask DMA is issued later (phase-3 program point) so it
                # does not compete with the critical startup transfers.
                ladj8 = apool.tile([128, 2, NT, N], F8E4, tag=f"ladj{g}",
                                   name="ladjz")
                if g == 0:
                    nc.gpsimd.dma_start(out=ladj8[:, 0], in_=d_ladj8[g])
                amrow = rpool.tile([1, N], F32, tag="gin2kf", bufs=4,
                                   name="amrow")
                nc.gpsimd.dma_start(out=amrow, in_=d_amask[g : g + 1, :])
                pmrow = rpool.tile([1, L], F32, tag="gin4kf", bufs=4,
                                   name="pmrow")
                nc.gpsimd.dma_start(out=pmrow, in_=d_pmask[g : g + 1, :])
                g_in.append((rows_in[g][0], rows_in[g][1], ladj8, amrow,
                             pmrow))
            nc.scalar.dma_start(out=blob16[:, 708:1484],
                                in_=d_blob16[:, 708:1484])
            if not CONV_FP8:
                # conv L0 weights early; L1/L2 issued at the phase-3 point
                nc.scalar.dma_start(out=blob16[:, 1484:2892],
                                    in_=d_blob16[:, 1484:2892])
            if CONV_FP8:
                nc.scalar.dma_start(out=blob8[:, MIT8O:],
                                    in_=d_blob8[:, MIT8O:])

            def late_dmas():
                nc.scalar.dma_start(out=g_in[1][2][:, 0], in_=d_ladj8[1])
                if not CONV_FP8:
                    nc.sync.dma_start(out=blob16[:, 2892:5708],
                                      in_=d_blob16[:, 2892:5708])

            def w16(nm, n, rows=128):
                return blob16[0:rows, W16O[nm] : W16O[nm] + n]

            Eat = w16("Eat", CD)
            Eam = w16("Eam", PD, NAM)
            Wa2h = w16("Wa2h", H)
            Wgoa2 = w16("Wgoa2", 2)
            E2 = w16("E2", 128, 33)
            Wc = w16("Wc", LAT)
            Wa = w16("Wa", LAT)
            Wg_flat = w16("Wg", H * GD)
            wa1c0 = blobf[:, WFO["wa1c"] : WFO["wa1c"] + 1]
            wa1c1 = blobf[:, WFO["wa1c"] + 1 : WFO["wa1c"] + 2]
            bc = blobf[0:LAT, WFO["bc"] : WFO["bc"] + 1]
            ba = blobf[0:LAT, WFO["ba"] : WFO["ba"] + 1]
            pw = blobf[0:LAT, WFO["pw"] : WFO["pw"] + 2]
            pb = blobf[0:1, WFO["pb"] : WFO["pb"] + 1]

            def MiT_v(lyr, i):
                off = W16O["MiT"] + (lyr * KW + i) * PD
                return blob16[:, off : off + PD]

            def Wgo_v(c):
                off = W16O["Wgo"] + c * CD
                return blob16[:, off : off + CD]

            def MiT8_v(lyr, pr):
                off = MIT8O + (lyr * NPR + pr) * 2 * PD
                return bass.AP(tensor=blob8.tensor,
                               offset=blob8.offset + off,
                               ap=[blob8.ap[0], [PD, 2], [1, PD]])

            def zlhs_v(li):
                # [128, 2, 128] DR lhsT: plane0 = ZBIG*I, plane1 = src-maker
                off = li * 2 * 128
                return bass.AP(tensor=blob8.tensor,
                               offset=blob8.offset + off,
                               ap=[blob8.ap[0], [128, 2], [1, 128]])

            def ladjz_rhs(g, t):
                # [128, 2, N] DR rhs: plane0 = A01 chunk t, plane1 = per-map
                lz = g_in[g][2]
                return bass.AP(tensor=lz.tensor,
                               offset=lz.offset + t * N,
                               ap=[lz.ap[0], [NT * N, 2], [1, N]])

            def leaky_act(out, in_, alpha, bias=None, accum_out=None):
                if use_prelu:
                    kw = {}
                    if bias is not None:
                        kw["bias"] = bias
                    if accum_out is not None:
                        kw["accum_out"] = accum_out
                    nc.scalar.activation(out=out, in_=in_, func=AF.Prelu,
                                         alpha=alpha, **kw)
                    return
                src = in_
                if bias is not None:
                    t = wpool.tile(list(out.shape), F32, tag="lk_t", bufs=4,
                                   name="lkb")
                    nc.scalar.activation(out=t, in_=in_, func=AF.Identity,
                                         bias=bias)
                    src = t
                nc.vector.scalar_tensor_tensor(
                    out=out, in0=src, scalar=alpha, in1=src,
                    op0=AT.mult, op1=AT.max)
                if accum_out is not None:
                    scr = wpool.tile(list(out.shape), F32, tag="lk_t2",
                                     bufs=4, name="lks")
                    nc.vector.tensor_scalar(out=scr, in0=out, scalar1=1.0,
                                            scalar2=0.0, op0=AT.mult,
                                            op1=AT.add, accum_out=accum_out)

            def dump_t(nm, src_ap, shape):
                if not dump:
                    return
                t = wpool.tile(shape, F32, tag="dumpbuf", bufs=2,
                               name="dump" + nm)
                nc.vector.tensor_copy(t, src_ap)
                dst = d_dbg[nm]
                sl = tuple([slice(None)] * len(shape))
                nc.sync.dma_start(out=dst[sl], in_=t)

            st = [dict() for _ in range(G)]

            # ================== phase 1: embeddings ==================
            for g in range(G):
                arow, prow, ladj8, amrow, pmrow = g_in[g]
                # atom one-hot -> avT [CD, N] (broadcast via PE)
                ab_ps = pssq.tile([128, N], F32, tag="sq", name="ab_ps")
                nc.tensor.matmul(ab_ps, onesrow, arow, start=True, stop=True)
                ohA = wpool.tile([128, N], MD, tag="t1k", bufs=6, name="ohA")
                nc.vector.tensor_tensor(out=ohA, in0=ab_ps, in1=iof[:, :N],
                                        op=AT.is_equal)
                avT_ps = pssq.tile([128, N], F32, tag="sq", name="avT_ps")
                nc.tensor.matmul(avT_ps, Eat, ohA, start=True, stop=True)
                avT = wpool.tile([128, N], MD, tag=f"avT{g}", bufs=1,
                                 name="avT")
                nc.scalar.copy(avT, avT_ps)
                st[g]["avT"] = avT

                # amino one-hot -> padded pv
                # dual-plane fp8 pv: plane1[k] = plane0[k+1] so DoubleRow
                # i-pairs read k-tiles at a 16B-aligned plane stride
                if CONV_FP8:
                    pv = apool.tile([PD, 2, PV8W], F8E4, tag=f"pv{g}_0",
                                    bufs=1, name="pv")
                    nc.vector.memset(pv[:, 0, :PADL], 0.0)
                    nc.vector.memset(pv[:, 0, PADL + L :], 0.0)
                    nc.vector.memset(pv[:, 1, : PADL - 1], 0.0)
                    nc.vector.memset(pv[:, 1, PADL - 1 + L :], 0.0)
                else:
                    pv = apool.tile([PD, PVW], MD, tag=f"pv{g}_0", bufs=1,
                                    name="pv")
                    nc.vector.memset(pv[:, :PADL], 0.0)
                    nc.vector.memset(pv[:, PADL + L :], 0.0)
                for c in range(2):
                    pb_ps = pssq.tile([128, N], F32, tag="sq", name="pb_ps")
                    nc.tensor.matmul(pb_ps, onesrow,
                                     prow[:, c * 512 : (c + 1) * 512],
                                     start=True, stop=True)
                    ohP = wpool.tile([NAM, N], MD, tag="t1k", bufs=6,
                                     name="ohP")
                    nc.vector.tensor_tensor(
                        out=ohP, in0=pb_ps[:NAM, :],
                        in1=iof[:NAM, c * 512 : (c + 1) * 512],
                        op=AT.is_equal)
                    pvT_ps = pscv.tile([PD, 512], F32, tag="cv",
                                       name="pvT_ps")
                    nc.tensor.matmul(pvT_ps, Eam, ohP, start=True, stop=True)
                    if CONV_FP8:
                        nc.scalar.copy(
                            pv[:, 0, PADL + c * 512 : PADL + (c + 1) * 512],
                            pvT_ps)
                        nc.vector.tensor_copy(
                            pv[:, 1, PADL - 1 + c * 512 :
                               PADL - 1 + (c + 1) * 512],
                            pv[:, 0, PADL + c * 512 : PADL + (c + 1) * 512])
                    else:
                        nc.scalar.copy(
                            pv[:, PADL + c * 512 : PADL + (c + 1) * 512],
                            pvT_ps)
                st[g]["pv"] = pv

            # conv machinery + L1 early (warms the PE during phase 2)
            # ================== conv machinery ==================
            conv_steps = []

            def make_conv_layer(lyr):
                pvo_l, cv = [], {}
                last = lyr == LC - 1
                for g in range(G):
                    if last:
                        pvo = apool.tile([PD, PVW], MD, tag=f"pvf{g}",
                                         bufs=1, name="pvo")
                    elif CONV_FP8:
                        pvo = apool.tile([PD, 2, PV8W], F8E4,
                                         tag=f"pv{g}_{1 - lyr % 2}", bufs=1,
                                         name="pvo")
                    else:
                        pvo = apool.tile([PD, PVW], MD,
                                         tag=f"pv{g}_{1 - lyr % 2}", bufs=1,
                                         name="pvo")
                    pvo_l.append(pvo)
                steps = []

                def mk_group(g, c):
                    def run():
                        if c == 0:
                            if last or not CONV_FP8:
                                nc.vector.memset(pvo_l[g][:, :PADL], 0.0)
                                nc.vector.memset(pvo_l[g][:, PADL + L :],
                                                 0.0)
                            else:
                                nc.vector.memset(pvo_l[g][:, 0, :PADL], 0.0)
                                nc.vector.memset(
                                    pvo_l[g][:, 0, PADL + L :], 0.0)
                                nc.vector.memset(
                                    pvo_l[g][:, 1, : PADL - 1], 0.0)
                                nc.vector.memset(
                                    pvo_l[g][:, 1, PADL - 1 + L :], 0.0)
                        cv[g] = pscv.tile([PD, 512], F32, tag="cv",
                                          name=f"cv{g}")
                        pv = st[g]["pv"]
                        if CONV_FP8:
                            for pr in range(NPR):
                                nc.tensor.matmul(
                                    cv[g], MiT8_v(lyr, pr),
                                    bass.AP(tensor=pv.tensor,
                                            offset=pv.offset + c * 512
                                            + 2 * pr,
                                            ap=[pv.ap[0], [PV8W, 2],
                                                [1, 512]]),
                                    start=(pr == 0), stop=(pr == NPR - 1),
                                    perf_mode=DR)
                        else:
                            for i in range(KW):
                                nc.tensor.matmul(
                                    cv[g], MiT_v(lyr, i),
                                    pv[:, c * 512 + i : c * 512 + i + 512],
                                    start=(i == 0), stop=(i == KW - 1))
                    return run

                def mk_relu(g, c):
                    def run():
                        if last or not CONV_FP8:
                            nc.vector.tensor_scalar(
                                out=pvo_l[g][:, PADL + c * 512 :
                                             PADL + (c + 1) * 512],
                                in0=cv[g], scalar1=cb[:, lyr : lyr + 1],
                                scalar2=0.0, op0=AT.add, op1=AT.max)
                        else:
                            nc.vector.tensor_scalar(
                                out=pvo_l[g][:, 0, PADL + c * 512 :
                                             PADL + (c + 1) * 512],
                                in0=cv[g], scalar1=cb[:, lyr : lyr + 1],
                                scalar2=0.0, op0=AT.add, op1=AT.max)
                            nc.vector.tensor_copy(
                                pvo_l[g][:, 1, PADL - 1 + c * 512 :
                                         PADL - 1 + (c + 1) * 512],
                                pvo_l[g][:, 0, PADL + c * 512 :
                                         PADL + (c + 1) * 512])
                    return run

                for c in range(2):
                    for g in range(G):
                        steps.append(mk_group(g, c))
                        steps.append(mk_relu(g, c))

                def finish():
                    for g in range(G):
                        st[g]["pv"] = pvo_l[g]
                    if dump and lyr == LC - 1:
                        dump_t("pv3", pvo_l[0], [128, PVW])
                steps.append(finish)
                return steps

            for lyr in range(LC):
                conv_steps.extend(make_conv_layer(lyr))

            # prot head rides the conv queue: each chunk is ready as soon
            # as the final conv relu for it has run
            opacc = {}

            def mk_prot(g, c):
                def run():
                    pv = st[g]["pv"]
                    pvt = psz.tile([128, 2, N], F32, tag="z", name="pv_ps")
                    pv_ps = pvt[:LAT, 0, :]
                    nc.tensor.matmul(pv_ps, Wa,
                                     pv[:, PADL + c * 512 :
                                        PADL + (c + 1) * 512],
                                     start=True, stop=True)
                    p_v = wpool.tile([LAT, 512], MD, tag="p_v", bufs=4,
                                     name="p_v")
                    pacc = rpool.tile([LAT, 1], F32, tag="c1", bufs=8,
                                      name="pacc")
                    leaky_act(p_v, pv_ps, ALPHA, bias=ba, accum_out=pacc)
                    opacc.setdefault(g, []).append(pacc)
                return run

            for c in range(2):
                for g in range(G):
                    conv_steps.append(mk_prot(g, c))

            def pop_conv(k):
                for _ in range(k):
                    if conv_steps:
                        conv_steps.pop(0)()


            # ================== phase 2: per-graph prep ==================
            for g in range(G):
                avT = st[g]["avT"]
                # all heads' Wh -> whsb_all[p, t, h, 0:64]; col 64 = 1.0
                whsb_all = wpool.tile([128, NT, H, GD + 2], MD,
                                      tag=f"whsb{g}", bufs=1, name="whsb_all")
                nc.vector.memset(whsb_all[:, :, :, GD : GD + 1], 1.0)
                for half in range(2):
                    wh_all = pssq.tile([128, 2, H * GD], F32, tag="sq",
                                       name="wh_all")
                    for t2 in range(2):
                        t = half * 2 + t2
                        nc.tensor.matmul(
                            wh_all[:, t2, :],
                            avT[:, t * 128 : (t + 1) * 128],
                            Wg_flat, start=True, stop=True)
                    nc.scalar.copy(
                        out=bass.AP(
                            tensor=whsb_all.tensor,
                            offset=whsb_all.offset
                            + half * 2 * H * (GD + 2),
                            ap=[whsb_all.ap[0], [H * (GD + 2), 2],
                                [GD + 2, H], [1, GD]]),
                        in_=wh_all)
                st[g]["whsb"] = whsb_all

                # dst bias columns for all heads: dstT[j, t*H+h] =
                # a2_h.Wh_j - ZBIG (the -ZBIG cancels plane0's ZBIG*A01)
                dst_ps = pssq.tile([128, NT * H], F32, tag="sq",
                                   name="dst_ps")
                for t in range(NT):
                    nc.tensor.matmul(dst_ps[:, t * H : (t + 1) * H],
                                     avT[:, t * 128 : (t + 1) * 128], Wa2h,
                                     start=True, stop=True)
                dstT = rpool.tile([128, NT * H], F32, tag=f"dstT{g}", bufs=1,
                                  name="dstT")
                nc.vector.tensor_scalar(out=dstT, in0=dst_ps, scalar1=-ZBIG,
                                        scalar2=None, op0=AT.add)
                st[g]["dstT"] = dstT
                # z rhs plane1 for head maps: avT in fp8e4
                ladjz = g_in[g][2]
                for t in range(NT):
                    nc.vector.tensor_copy(ladjz[:, 1, t, :], avT)


            # ============ attention z/U (per head or out-layer) ============
            def att_zU(g, li, bias4, dump_zu=False):
                """One DR matmul per chunk: z = ZBIG*A01 + src; dst - ZBIG
                enters as the Prelu bias. Returns U [128, NT, N] fp16."""
                ee = apool.tile([128, NT, N], MD, tag="ee", bufs=2, name="ee")
                for th in range(2):
                    zps = psz.tile([128, 2, N], F32, tag="z", name="zps")
                    for t2 in range(2):
                        t = th * 2 + t2
                        nc.tensor.matmul(zps[:, t2, :], zlhs_v(li),
                                         ladjz_rhs(g, t), start=True,
                                         stop=True, perf_mode=DR)
                    for t2 in range(2):
                        t = th * 2 + t2
                        if use_prelu:
                            nc.scalar.activation(out=ee[:, t, :],
                                                 in_=zps[:, t2, :],
                                                 func=AF.Prelu, alpha=ALPHA,
                                                 bias=bias4[t])
                        else:
                            eesc = wpool.tile([128, N], F32, tag="eesc",
                                              bufs=4, name="eesc")
                            nc.scalar.activation(out=eesc, in_=zps[:, t2, :],
                                                 func=AF.Identity,
                                                 bias=bias4[t])
                            nc.vector.scalar_tensor_tensor(
                                out=ee[:, t, :], in0=eesc, scalar=ALPHA,
                                in1=eesc, op0=AT.mult, op1=AT.max)
                if dump_zu:
                    dump_t("zm0", ee, [128, NT, N])
                U = apool.tile([128, NT, N], MD, tag="U", bufs=2, name="U")
                nc.scalar.activation(out=U, in_=ee, func=AF.Exp)
                if dump_zu:
                    dump_t("U0", U, [128, NT, N])
                return U

            def elu_norm_pair(hp0, hp1, dst):
                """Normalize two heads' hp [65, N] (row 64 = rowsum) and
                elu into packed dst [128, N]."""
                rsum2 = rpool.tile([33, N], F32, tag="zrw2", bufs=3,
                                   name="rsum2")
                nc.gpsimd.memset(rsum2, 1.0)
                nc.vector.tensor_copy(rsum2[0:1, :], hp0[64:65, :])
                nc.vector.tensor_copy(rsum2[32:33, :], hp1[64:65, :])
                rinv2 = rpool.tile([33, N], F32, tag="zrw2", bufs=3,
                                   name="rinv2")
                nc.vector.reciprocal_approx_fast(out=rinv2, in_=rsum2)
                rinv16 = rpool.tile([33, N], MD, tag="ri16", bufs=2,
                                    name="rinv16")
                nc.vector.tensor_scalar(out=rinv16, in0=rinv2, scalar1=1.0,
                                        scalar2=None, op0=AT.mult)
                rb_ps = pssq.tile([128, N], F32, tag="sq", name="rb_ps")
                nc.tensor.matmul(rb_ps, E2, rinv16, start=True, stop=True)
                rb = wpool.tile([128, N], MD, tag="rb", bufs=2, name="rb")
                nc.vector.tensor_copy(rb, rb_ps)
                hpn = wpool.tile([128, N], MD, tag="elu", bufs=4, name="hpn")
                nc.vector.scalar_tensor_tensor(
                    out=hpn[0:64, :], in0=hp0[:64, :], scalar=1.0,
                    in1=rb[0:64, :], op0=AT.mult, op1=AT.mult)
                nc.vector.scalar_tensor_tensor(
                    out=hpn[64:128, :], in0=hp1[:64, :], scalar=1.0,
                    in1=rb[64:128, :], op0=AT.mult, op1=AT.mult)
                xm = wpool.tile([128, N], MD, tag="elu", bufs=4, name="xm")
                nc.vector.tensor_scalar(out=xm, in0=hpn, scalar1=0.0,
                                        scalar2=None, op0=AT.min)
                em = wpool.tile([128, N], MD, tag="elu", bufs=4, name="em")
                nc.scalar.activation(out=em, in_=xm, func=AF.Exp)
                nc.vector.scalar_tensor_tensor(
                    out=dst, in0=em, scalar=-1.0, in1=hpn,
                    op0=AT.add, op1=AT.max)

            # ================== phase 3: head maps + conv ==================
            for g in range(G):
                m01 = wpool.tile([128, N], MD, tag=f"m01{g}", bufs=1,
                                 name="m01")
                m23 = wpool.tile([128, N], MD, tag=f"m23{g}", bufs=1,
                                 name="m23")
                st[g]["multi"] = (m01, m23)
                st[g]["hp"] = {}

            # software-pipelined attention: emit z/U of map k+1 before the
            # hp matmuls of map k so the PE never stalls on the exp chain
            pend = None          # (g, h, U) waiting for its hp emission
            pair_hps = {}

            def emit_hp(g, h, U):
                whsb_all = st[g]["whsb"]
                hp_t = pshp.tile([128, N], F32, tag="hp", name="hp_ps")
                hp = hp_t[0:65, :]
                for t in range(NT):
                    nc.tensor.matmul(hp, whsb_all[:, t, h, : GD + 1],
                                     U[:, t, :],
                                     start=(t == 0), stop=(t == NT - 1))
                pair_hps.setdefault(g, []).append(hp)
                if len(pair_hps[g]) == 2:
                    hps = pair_hps.pop(g)
                    elu_norm_pair(hps[0], hps[1], st[g]["multi"][h // 2])

            # --- per-graph tail chunks (pipelined into later head maps) ---
            oU, ohp, orb = {}, {}, {}

            def tail_a1(g):
                m01, m23 = st[g]["multi"]
                multi = [m01, m23]
                wh2_ps = pssq.tile([128, NT, CD], F32, tag="sq",
                                   name="wh2_ps")
                for t in range(NT):
                    for c in range(2):
                        nc.tensor.matmul(
                            wh2_ps[:, t, :],
                            multi[c][:, t * 128 : (t + 1) * 128],
                            Wgo_v(c), start=(c == 0), stop=(c == 1))
                wh2sb = wpool.tile([128, NT, CD], MD, tag=f"wh2{g}", bufs=1,
                                   name="wh2sb")
                for t in range(NT):
                    nc.vector.tensor_copy(wh2sb[:, t, :], wh2_ps[:, t, :])
                st[g]["wh2sb"] = wh2sb
                # out-layer z rhs plane1: q[k,i] = a1go[k,0]*m01 + a1go[k,1]
                # *m23 (ones-lhsT row-sums it into src2)
                qt1 = wpool.tile([128, N], MD, tag="qt", bufs=4, name="qt1")
                nc.vector.tensor_scalar(out=qt1, in0=m01, scalar1=wa1c0,
                                        scalar2=None, op0=AT.mult)
                qt2 = wpool.tile([128, N], MD, tag="qt", bufs=4, name="qt2")
                nc.vector.tensor_scalar(out=qt2, in0=m23, scalar1=wa1c1,
                                        scalar2=None, op0=AT.mult)
                ladjz = g_in[g][2]
                for t in range(NT):
                    nc.vector.tensor_tensor(out=ladjz[:, 1, t, :], in0=qt1,
                                            in1=qt2, op=AT.add)
                # out-layer dst bias columns: dstT2[j, t] = a2go.xo_j - ZBIG
                d2_ps = pssq.tile([128, NT], F32, tag="sq", name="d2_ps")
                for t in range(NT):
                    for c in range(2):
                        nc.tensor.matmul(
                            d2_ps[:, t : t + 1],
                            multi[c][:, t * 128 : (t + 1) * 128],
                            Wgoa2[:, c : c + 1],
                            start=(c == 0), stop=(c == 1))
                dstT2 = rpool.tile([128, NT], F32, tag=f"d2T{g}", bufs=1,
                                   name="dstT2")
                nc.vector.tensor_scalar(out=dstT2, in0=d2_ps, scalar1=-ZBIG,
                                        scalar2=None, op0=AT.add)
                st[g]["dstT2"] = dstT2

            def tail_out(g):
                # out-layer map + aggregation, per-chunk pipelined
                dstT2 = st[g]["dstT2"]
                wh2sb = st[g]["wh2sb"]
                hp_t = pshp.tile([128, N], F32, tag="hp", name="hp2_ps")
                rst = pshp.tile([128, N], F32, tag="hp", name="rs_ps")
                Uo = apool.tile([128, NT, N], MD, tag="U", bufs=2, name="Uo")
                for th in range(2):
                    zps = psz.tile([128, 2, N], F32, tag="z", name="zps")
                    for t2 in range(2):
                        t = th * 2 + t2
                        nc.tensor.matmul(zps[:, t2, :], zlhs_v(H),
                                         ladjz_rhs(g, t), start=True,
                                         stop=True, perf_mode=DR)
                    for t2 in range(2):
                        t = th * 2 + t2
                        ee1 = wpool.tile([128, N], MD, tag="eeo", bufs=4,
                                         name="ee1")
                        if use_prelu:
                            nc.scalar.activation(out=ee1, in_=zps[:, t2, :],
                                                 func=AF.Prelu, alpha=ALPHA,
                                                 bias=dstT2[:, t : t + 1])
                        else:
                            eesc = wpool.tile([128, N], F32, tag="eesc",
                                              bufs=4, name="eesc")
                            nc.scalar.activation(out=eesc, in_=zps[:, t2, :],
                                                 func=AF.Identity,
                                                 bias=dstT2[:, t : t + 1])
                            nc.vector.scalar_tensor_tensor(
                                out=ee1, in0=eesc, scalar=ALPHA,
                                in1=eesc, op0=AT.mult, op1=AT.max)
                        nc.scalar.activation(out=Uo[:, t, :], in_=ee1,
                                             func=AF.Exp)
                        nc.tensor.matmul(hp_t, wh2sb[:, t, :], Uo[:, t, :],
                                         start=(t == 0), stop=(t == NT - 1))
                        nc.tensor.matmul(rst[0:1, :], ones16c[:, 0:1],
                                         Uo[:, t, :],
                                         start=(t == 0), stop=(t == NT - 1))
                rinv = rpool.tile([1, N], F32, tag="zrwo", bufs=4,
                                  name="rinvo")
                nc.vector.reciprocal_approx_fast(out=rinv, in_=rst[0:1, :])
                rinv16 = rpool.tile([1, N], MD, tag="ri16o", bufs=2,
                                    name="rinv16o")
                nc.vector.tensor_scalar(out=rinv16, in0=rinv, scalar1=1.0,
                                        scalar2=None, op0=AT.mult)
                rb_ps = pssq.tile([128, N], F32, tag="sq", name="rb2_ps")
                nc.tensor.matmul(rb_ps, onesrow, rinv16, start=True,
                                 stop=True)
                rb = wpool.tile([128, N], MD, tag="rb", bufs=2, name="rbo")
                nc.vector.tensor_copy(rb, rb_ps)
                ohp[g] = hp_t
                orb[g] = rb

            def tail_b(g):
                # xT = elu(hp2 / rowsum), then comp head + pooling
                xT = wpool.tile([CD, N], MD, tag=f"xT{g}", bufs=1, name="xT")
                hpn = wpool.tile([128, N], MD, tag="elu", bufs=4,
                                 name="hpno")
                nc.vector.scalar_tensor_tensor(
                    out=hpn, in0=ohp[g], scalar=1.0, in1=orb[g],
                    op0=AT.mult, op1=AT.mult)
                xm = wpool.tile([128, N], MD, tag="elu", bufs=4, name="xmo")
                nc.vector.tensor_scalar(out=xm, in0=hpn, scalar1=0.0,
                                        scalar2=None, op0=AT.min)
                em = wpool.tile([128, N], MD, tag="elu", bufs=4, name="emo")
                nc.scalar.activation(out=em, in_=xm, func=AF.Exp)
                nc.vector.scalar_tensor_tensor(
                    out=xT, in0=em, scalar=-1.0, in1=hpn,
                    op0=AT.add, op1=AT.max)
                av_t = pshp.tile([128, N], F32, tag="hp", name="av_ps")
                av = av_t[:LAT, :]
                nc.tensor.matmul(av, Wc, xT, start=True, stop=True)
                avec = wpool.tile([LAT, N], MD, tag="avec", bufs=2,
                                  name="avec")
                leaky_act(avec, av, ALPHA, bias=bc)
                av2_t = pshp.tile([128, N], F32, tag="hp", name="av2_ps")
                av2 = av2_t[:LAT, :]
                nc.tensor.matmul(av2, Wa, avec, start=True, stop=True)
                a_v = wpool.tile([LAT, N], MD, tag="avec", bufs=2,
                                 name="a_v")
                comp_acc = rpool.tile([LAT, 1], F32, tag="c1", bufs=8,
                                      name="comp_acc")
                leaky_act(a_v, av2, ALPHA, bias=ba, accum_out=comp_acc)
                amrow = g_in[g][3]
                amscr = rpool.tile([1, N], F32, tag="r2k", bufs=6,
                                   name="amscr")
                amsum = rpool.tile([1, 1], F32, tag="c2", bufs=12,
                                   name="amsum")
                nc.vector.tensor_scalar(out=amscr, in0=amrow, scalar1=1.0,
                                        scalar2=0.0, op0=AT.mult, op1=AT.add,
                                        accum_out=amsum)
                amsb = rpool.tile([128, 1], F32, tag="c2", bufs=12,
                                  name="amsb")
                nc.gpsimd.partition_broadcast(amsb, amsum)
                amr = rpool.tile([128, 1], F32, tag="c2", bufs=12,
                                 name="amr")
                nc.vector.reciprocal(amr, amsb)
                cp = rpool.tile([128, 2], F32, tag="cp", bufs=4, name="cp")
                nc.vector.tensor_scalar(out=cp[:, 0:1], in0=comp_acc,
                                        scalar1=amr, scalar2=None,
                                        op0=AT.mult)
                st[g]["cp"] = cp

            def tail_c(g):
                # prot pooling + prediction (opacc filled via the conv queue)
                cp = st[g]["cp"]
                prot_acc = rpool.tile([LAT, 1], F32, tag="c1", bufs=8,
                                      name="prot_acc")
                nc.vector.tensor_tensor(out=prot_acc, in0=opacc[g][0],
                                        in1=opacc[g][1], op=AT.add)
                pmrow = g_in[g][4]
                pmscr = rpool.tile([1, L], F32, tag="r4k", bufs=2,
                                   name="pmscr")
                pmsum = rpool.tile([1, 1], F32, tag="c2", bufs=12,
                                   name="pmsum")
                nc.vector.tensor_scalar(out=pmscr, in0=pmrow, scalar1=1.0,
                                        scalar2=0.0, op0=AT.mult, op1=AT.add,
                                        accum_out=pmsum)
                pmsb = rpool.tile([128, 1], F32, tag="c2", bufs=12,
                                  name="pmsb")
                nc.gpsimd.partition_broadcast(pmsb, pmsum)
                pmr = rpool.tile([128, 1], F32, tag="c2", bufs=12,
                                 name="pmr")
                nc.vector.reciprocal(pmr, pmsb)
                nc.vector.tensor_scalar(out=cp[:, 1:2], in0=prot_acc,
                                        scalar1=pmr, scalar2=None,
                                        op0=AT.mult)
                lr2 = rpool.tile([128, 2], F32, tag="cp", bufs=4, name="lr2")
                leaky_act(lr2, cp, ALPHA * ALPHA)
                dscr = rpool.tile([128, 2], F32, tag="cp", bufs=4,
                                  name="dscr")
                dacc = rpool.tile([128, 1], F32, tag="c1", bufs=8,
                                  name="dacc")
                nc.vector.scalar_tensor_tensor(
                    out=dscr, in0=lr2, scalar=1.0, in1=pw,
                    op0=AT.mult, op1=AT.mult, accum_out=dacc)
                fin_ps = pssq.tile([128, N], F32, tag="sq", name="fin_ps")
                nc.tensor.matmul(fin_ps[0:1, 0:1], dacc, ones_col,
                                 start=True, stop=True)
                res = rpool.tile([1, 1], F32, tag="c2", bufs=12, name="res")
                nc.scalar.activation(out=res, in_=fin_ps[0:1, 0:1],
                                     func=AF.Identity, bias=pb)
                nc.sync.dma_start(out=d_out[g : g + 1, :], in_=res)

            # schedule: g-major maps (pipelined); barrier tails afterwards,
            # g1's out-layer PE work overlapping g0's serial pooling chain
            pop_conv(2)
            for g in range(G):
                dstT = st[g]["dstT"]
                for h in range(H):
                    U = att_zU(g, h,
                               [dstT[:, t * H + h : t * H + h + 1]
                                for t in range(NT)],
                               dump_zu=(dump and g == 0 and h == 0))
                    if g == 0 and h == 0:
                        late_dmas()
                    if pend is not None:
                        emit_hp(*pend)
                    pend = (g, h, U)
                    pop_conv(2)
            if pend is not None:
                emit_hp(*pend)
                pend = None
            tail_a1(0)
            pop_conv(3)
            tail_a1(1)
            pop_conv(3)
            tail_out(0)
            pop_conv(4)
            tail_b(0)
            pop_conv(len(conv_steps))
            tail_out(1)
            tail_b(1)
            tail_c(0)
            tail_c(1)

    return nc


def preprocess(inputs, mm_bf16=False):
    """Host-side prep: shard over cores, transpose/reshape weights."""
    import ml_dtypes
    md = np.float16
    f8 = ml_dtypes.float8_e4m3fn
    atoms = np.asarray(inputs["atoms"]).astype(np.float32)
    atoms_mask = np.asarray(inputs["atoms_mask"]).astype(np.float32)
    adjacency = np.asarray(inputs["adjacency"])
    amino = np.asarray(inputs["amino"]).astype(np.float32)
    amino_mask = np.asarray(inputs["amino_mask"]).astype(np.float32)
    E_atom = np.asarray(inputs["E_atom"]).astype(np.float32)
    E_amino = np.asarray(inputs["E_amino"]).astype(np.float32)
    W_gat = np.asarray(inputs["W_gat"]).astype(np.float32)
    a_gat = np.asarray(inputs["a_gat"]).astype(np.float32)
    W_go = np.asarray(inputs["W_go"]).astype(np.float32)
    a_go = np.asarray(inputs["a_go"]).astype(np.float32)
    W_comp_w = np.asarray(inputs["W_comp_w"]).astype(np.float32)
    W_comp_b = np.asarray(inputs["W_comp_b"]).astype(np.float32)
    conv_w = np.asarray(inputs["conv_w"]).astype(np.float32)
    conv_b = np.asarray(inputs["conv_b"]).astype(np.float32)
    W_att_w = np.asarray(inputs["W_att_w"]).astype(np.float32)
    W_att_b = np.asarray(inputs["W_att_b"]).astype(np.float32)
    pred_w = np.asarray(inputs["pred_w"]).astype(np.float32)
    pred_b = np.asarray(inputs["pred_b"]).astype(np.float32)

    ladjT = (adjacency.transpose(0, 2, 1) > 0).astype(np.float32)
    ladjT_r = np.ascontiguousarray(
        ladjT.reshape(B, NT, 128, N).transpose(0, 2, 1, 3))

    E_atom_pad = np.zeros((128, CD), np.float32)
    E_atom_pad[:NA] = E_atom

    MiT = np.zeros((LC, KW, PD, PD), np.float32)
    din = np.arange(PD)[:, None]
    dout = np.arange(PD)[None, :]
    v = din - dout + (KW // 2)
    valid = (v >= 0) & (v < KW)
    vc = np.clip(v, 0, KW - 1)
    for lyr in range(LC):
        for i in range(KW):
            MiT[lyr, i] = np.where(valid, conv_w[lyr, 0, 0, i, vc], 0.0)
    MiT_r = np.ascontiguousarray(MiT.transpose(2, 0, 1, 3))
    NPRl = (KW + 1) // 2
    MiT8 = np.zeros((LC, NPRl, 2, PD, PD), np.float32)
    for lyr in range(LC):
        for pr in range(NPRl):
            MiT8[lyr, pr, 0] = MiT[lyr, 2 * pr]
            if 2 * pr + 1 < KW:
                MiT8[lyr, pr, 1] = MiT[lyr, 2 * pr + 1]
    MiT8_r = np.ascontiguousarray(MiT8.transpose(3, 0, 1, 2, 4))

    W_gat_r = np.ascontiguousarray(W_gat.transpose(1, 0, 2))
    Wa1h = np.einsum("hpq,hq->ph", W_gat, a_gat[:, :GD])
    Wa2h = np.einsum("hpq,hq->ph", W_gat, a_gat[:, GD:])

    W_go_r = np.ascontiguousarray(
        W_go.reshape(2, 128, CD).transpose(1, 0, 2))
    Wgo_a2 = (W_go @ a_go[CD:]).reshape(2, 128).T     # (128, 2)
    Wgo_a1 = (W_go @ a_go[:CD]).reshape(2, 128).T

    E2 = np.zeros((33, 128), np.float32)
    E2[0, 0:64] = 1.0
    E2[32, 64:128] = 1.0

    # pack fp16 weight blob (offsets mirror kernel W16O)
    W16N = 1484 if CONV_FP8 else 1484 + LC * KW * PD
    blob16 = np.zeros((128, W16N), np.float32)

    def put16(off, arr):
        a = np.asarray(arr, np.float32)
        blob16[: a.shape[0], off : off + a.shape[1]] = a

    put16(0, E_atom_pad)
    put16(128, E_amino)
    put16(256, W_gat_r.reshape(CD, H * GD))
    put16(512, Wa2h)
    put16(708, W_go_r.reshape(128, 2 * CD))
    put16(964, Wgo_a2)
    put16(1100, E2)
    put16(1228, W_comp_w.T)
    put16(1356, W_att_w.T)
    if not CONV_FP8:
        put16(1484, MiT_r.reshape(PD, LC * KW * PD))

    blobf = np.zeros((128, 7), np.float32)
    blobf[:, 0:2] = Wgo_a1
    blobf[:LAT, 2] = W_comp_b
    blobf[:LAT, 3] = W_att_b
    blobf[:LAT, 4] = pred_w[0, :LAT]
    blobf[:LAT, 5] = pred_w[0, LAT:]
    blobf[0, 6] = pred_b[0]

    # fp8 blob: z lhsT planes (ZBIG*I | src-maker) per map, then MiT8
    B8Wl = NZL * 2 * 128 + (LC * NPRl * 2 * PD if CONV_FP8 else 0)
    blob8 = np.zeros((128, B8Wl), np.float32)
    zlarr = np.zeros((128, NZL, 2, 128), np.float32)
    for li in range(NZL):
        zlarr[:, li, 0, :] = ZBIG * np.eye(128, dtype=np.float32)
    for h in range(H):
        zlarr[:, h, 1, :] = Wa1h[:, h : h + 1]
    zlarr[:, H, 1, :] = 1.0
    blob8[:, : NZL * 2 * 128] = zlarr.reshape(128, NZL * 2 * 128)
    if CONV_FP8:
        blob8[:, NZL * 2 * 128 :] = MiT8_r.reshape(PD, LC * NPRl * 2 * PD)

    shared = {
        "blob16": blob16.astype(md),
        "blobf": blobf,
        "blob8": blob8.astype(ml_dtypes.float8_e4m3fn),
        "conv_b": np.ascontiguousarray(conv_b.reshape(LC, 1)),
    }
    in_maps = []
    for c in range(NCORES):
        sl = slice(c * G, (c + 1) * G)
        m = dict(shared)
        m["atoms_f"] = np.ascontiguousarray(atoms[sl]).astype(md)
        m["atoms_mask"] = np.ascontiguousarray(atoms_mask[sl])
        m["ladj8"] = np.ascontiguousarray(ladjT_r[sl]).astype(f8)
        m["amino_f"] = np.ascontiguousarray(amino[sl]).astype(md)
        m["amino_mask"] = np.ascontiguousarray(amino_mask[sl])
        in_maps.append(m)
    return in_maps


_CACHED_NC = None


def kernel(**inputs) -> np.ndarray:
    global _CACHED_NC
    from concourse.bass_utils import run_bass_kernel_spmd

    if _CACHED_NC is None:
        nc = build_core_program()
        nc.finalize()
        _CACHED_NC = nc
    nc = _CACHED_NC
    in_maps = preprocess(inputs)
    res = run_bass_kernel_spmd(nc, in_maps, core_ids=list(range(NCORES)))
    out = np.concatenate([res.results[c]["out"] for c in range(NCORES)], axis=0)
    return out.astype(np.float32)



# revision 64
# speedup vs baseline: 1.1134x; 1.0948x over previous
"""Trainium2 Bass kernel for BiDACPI (GAT + CNN + bidirectional attention).

Data-parallel over batch b=16 across 8 NeuronCores (2 graphs per core).
Self-contained: hardcodes all shapes; host-side preprocessing only reshapes /
transposes weights and converts index tensors.

v3: attention logits z = mask + src_i + dst_j are built ON THE PE
(fp8-DoubleRow identity x mask matmul + K=2 rank-2 matmul of
[dst;ones]^T [ones;src]), eliminating the DVE broadcast/accumulator ops
that dominated v2. Heads are processed in pairs so softmax-normalize +
elu run on packed [128, 512] tiles. fp16 matmuls; f32 on DVE
scalar_tensor_tensor paths (fp16 is slower there); fp16 only where DVE
tensor_scalar/copy 4x modes apply. Pooling is folded into activation
accum_out (masks are spec'd fill=ones).
"""
import numpy as np

import concourse.bass as bass
import concourse.mybir as mybir
import concourse.tile as tile
from concourse import bacc

F32 = mybir.dt.float32
F16 = mybir.dt.float16
F8 = mybir.dt.float8e5
F8E4 = mybir.dt.float8e4
I32 = mybir.dt.int32
AT = mybir.AluOpType
AF = mybir.ActivationFunctionType
DR = mybir.MatmulPerfMode.DoubleRow

# Problem constants
B = 16
NCORES = 8
G = B // NCORES          # graphs per core
N = 512                  # atoms per graph
L = 1024                 # amino length
CD = 128                 # comp_dim
PD = 128                 # prot_dim
GD = 64                  # gat_dim
H = 4                    # heads
LAT = 128                # latent
NA = 100                 # num_atom
NAM = 30                 # num_amino
LC = 3                   # conv layers
KW = 11                  # conv kernel width
ALPHA = 0.2
MASKNEG = -28672.0       # fp8e5-exact additive mask
NT = N // 128            # 4 j-chunks
PADL = KW // 2
PVW = PADL + L + PADL + 2  # padded pv width (1036, even)

MASK_FP8_DR = True       # fp8e5 identity-mask matmul
LEAKY_SPLIT = 5          # unused
CONV_FP8 = False         # conv band matmuls in fp8e4m3 + DoubleRow i-pairs
NPR = (KW + 1) // 2      # 6 DoubleRow pairs
PV8W = 1040              # fp8 dual-plane width (16B-aligned plane stride)
ZBIG = 128.0             # adjacency scale: z = ZBIG*A01 + src - ZBIG + dst
NZL = 5                  # z lhsT planes: 4 heads + out layer
MIT8O = NZL * 2 * 128    # MiT8 offset in blob8 (zlhs first)
B8W = MIT8O + (LC * NPR * 2 * PD if CONV_FP8 else 0)


def build_core_program(debug=False, mm_bf16=False, dump=False):
    """debug=True builds the CoreSim-compatible variant (no Prelu; no
    activation accum_out)."""
    if debug:
        nc = bacc.Bacc(None, target_bir_lowering=False, debug=True)
    else:
        nc = bacc.Bacc(None)
    MD = F16
    use_prelu = not debug

    # ---- DRAM I/O ----
    d_atoms = nc.dram_tensor("atoms_f", [G, N], MD, kind="ExternalInput")
    d_amask = nc.dram_tensor("atoms_mask", [G, N], F32, kind="ExternalInput")
    # ladj8[g, p, t, i] = binary adjacency A01 for edge j=t*128+p -> i
    d_ladj8 = nc.dram_tensor("ladj8", [G, 128, NT, N], F8E4,
                             kind="ExternalInput")
    d_amino = nc.dram_tensor("amino_f", [G, L], MD, kind="ExternalInput")
    d_pmask = nc.dram_tensor("amino_mask", [G, L], F32, kind="ExternalInput")
    # packed weight blobs (one DMA each): offsets must match preprocess()
    W16O = dict(Eat=0, Eam=128, Wg=256, Wa2h=512, Wgo=708,
                Wgoa2=964, E2=1100, Wc=1228, Wa=1356, MiT=1484)
    W16N = 1484 if CONV_FP8 else 1484 + LC * KW * PD
    WFO = dict(wa1c=0, bc=2, ba=3, pw=4, pb=6)
    WFN = 7
    d_blob16 = nc.dram_tensor("blob16", [128, W16N], MD,
                              kind="ExternalInput")
    d_blobf = nc.dram_tensor("blobf", [128, WFN], F32, kind="ExternalInput")
    d_blob8 = nc.dram_tensor("blob8", [128, B8W], F8E4,
                             kind="ExternalInput")
    d_cb = nc.dram_tensor("conv_b", [LC, 1], F32, kind="ExternalInput")
    d_out = nc.dram_tensor("out", [G, 1], F32, kind="ExternalOutput")
    d_dbg = {}
    if dump:
        for nm, shp in [("U0", [128, NT, N]), ("zm0", [128, NT, N]),
                        ("m01", [128, N]), ("m23", [128, N]),
                        ("xT", [128, N]), ("rinv0", [1, N]),
                        ("pv3", [128, PVW]), ("comp", [LAT, 1]),
                        ("prot", [LAT, 1])]:
            d_dbg[nm] = nc.dram_tensor("dbg_" + nm, shp, F32,
                                       kind="ExternalOutput")

    with tile.TileContext(nc) as tc:
        with (
            tc.tile_pool(name="const", bufs=1) as cpool,
            tc.tile_pool(name="work", bufs=1) as wpool,
            tc.tile_pool(name="att", bufs=1) as apool,
            tc.tile_pool(name="rows", bufs=1) as rpool,
            tc.tile_pool(name="ps_sq", bufs=1, space="PSUM") as pssq,
            tc.tile_pool(name="ps_hp", bufs=2, space="PSUM") as pshp,
            tc.tile_pool(name="ps_z", bufs=2, space="PSUM") as psz,
            tc.tile_pool(name="ps_cv", bufs=1, space="PSUM") as pscv,
        ):
            # queue heads: iota first on gpsimd (gates one-hots), const
            # memsets first on DVE
            ioi = cpool.tile([128, L], I32)
            nc.gpsimd.iota(ioi, pattern=[[0, L]], base=0,
                           channel_multiplier=1)
            ones_col = cpool.tile([128, 1], F32)
            nc.vector.memset(ones_col, 1.0)
            ones16c = cpool.tile([128, 2], MD)
            nc.vector.memset(ones16c, 1.0)
            onesrow = cpool.tile([1, 128], MD)
            nc.vector.memset(onesrow, 1.0)
            iof = cpool.tile([128, L], F32)
            nc.vector.tensor_copy(iof, ioi)

            # ---- input DMAs (phase-1-critical first on each queue) ----
            g_in = []
            rows_in = []
            for g in range(G):
                arow = rpool.tile([1, N], MD, tag="gin1k", bufs=4,
                                  name="arow")
                nc.sync.dma_start(out=arow, in_=d_atoms[g : g + 1, :])
                prow = rpool.tile([1, L], MD, tag="gin2k", bufs=4,
                                  name="prow")
                nc.sync.dma_start(out=prow, in_=d_amino[g : g + 1, :])
                rows_in.append((arow, prow))

            # ---- weights: blob DMAs + cb; masks-first on scalar queue ----
            blob16 = cpool.tile([128, W16N], MD, tag="blob16", name="blob16")
            nc.sync.dma_start(out=blob16[:, :256], in_=d_blob16[:, :256])
            nc.sync.dma_start(out=blob16[:, 256:708], in_=d_blob16[:, 256:708])
            blobf = cpool.tile([128, WFN], F32, tag="blobf", name="blobf")
            nc.sync.dma_start(out=blobf, in_=d_blobf[:, :])
            blob8 = cpool.tile([128, B8W], F8E4, tag="blob8", name="blob8")
            nc.scalar.dma_start(out=blob8[:, :MIT8O], in_=d_blob8[:, :MIT8O])
            cb = cpool.tile([128, LC], F32)
            nc.gpsimd.dma_start(
                out=cb,
                in_=bass.AP(tensor=d_cb, offset=0,
                            ap=[[0, 128], [1, LC], [0, 1]]),
            )
            for g in range(G):
                # dual-plane rhs: [:, 0] = A01 (contiguous DMA), [:, 1, t, :]
                # = per-map plane (avT8 for heads / q8 for the out layer)
                ladj8 = apool.tile([128, 2, NT, N], F8E4, tag=f"ladj{g}",
                                   name="ladjz")
                if g == 0:
                    nc.gpsimd.dma_start(out=ladj8[:, 0], in_=d_ladj8[g])
                else:
                    nc.scalar.dma_start(out=ladj8[:, 0], in_=d_ladj8[g])
                amrow = rpool.tile([1, N], F32, tag="gin2kf", bufs=4,
                                   name="amrow")
                nc.gpsimd.dma_start(out=amrow, in_=d_amask[g : g + 1, :])
                pmrow = rpool.tile([1, L], F32, tag="gin4kf", bufs=4,
                                   name="pmrow")
                nc.gpsimd.dma_start(out=pmrow, in_=d_pmask[g : g + 1, :])
                g_in.append((rows_in[g][0], rows_in[g][1], ladj8, amrow,
                             pmrow))
            nc.scalar.dma_start(out=blob16[:, 708:1484],
                                in_=d_blob16[:, 708:1484])
            if not CONV_FP8:
                # conv L0 weights early; L1/L2 issued at the phase-3 point
                nc.scalar.dma_start(out=blob16[:, 1484:2892],
                                    in_=d_blob16[:, 1484:2892])

            def late_dmas():
                if not CONV_FP8:
                    nc.sync.dma_start(out=blob16[:, 2892:5708],
                                      in_=d_blob16[:, 2892:5708])
            if CONV_FP8:
                nc.scalar.dma_start(out=blob8[:, MIT8O:],
                                    in_=d_blob8[:, MIT8O:])

            def w16(nm, n, rows=128):
                return blob16[0:rows, W16O[nm] : W16O[nm] + n]

            Eat = w16("Eat", CD)
            Eam = w16("Eam", PD, NAM)
            Wa2h = w16("Wa2h", H)
            Wgoa2 = w16("Wgoa2", 2)
            E2 = w16("E2", 128, 33)
            Wc = w16("Wc", LAT)
            Wa = w16("Wa", LAT)
            Wg_flat = w16("Wg", H * GD)
            wa1c0 = blobf[:, WFO["wa1c"] : WFO["wa1c"] + 1]
            wa1c1 = blobf[:, WFO["wa1c"] + 1 : WFO["wa1c"] + 2]
            bc = blobf[0:LAT, WFO["bc"] : WFO["bc"] + 1]
            ba = blobf[0:LAT, WFO["ba"] : WFO["ba"] + 1]
            pw = blobf[0:LAT, WFO["pw"] : WFO["pw"] + 2]
            pb = blobf[0:1, WFO["pb"] : WFO["pb"] + 1]

            def MiT_v(lyr, i):
                off = W16O["MiT"] + (lyr * KW + i) * PD
                return blob16[:, off : off + PD]

            def Wgo_v(c):
                off = W16O["Wgo"] + c * CD
                return blob16[:, off : off + CD]

            def MiT8_v(lyr, pr):
                off = MIT8O + (lyr * NPR + pr) * 2 * PD
                return bass.AP(tensor=blob8.tensor,
                               offset=blob8.offset + off,
                               ap=[blob8.ap[0], [PD, 2], [1, PD]])

            def zlhs_v(li):
                # [128, 2, 128] DR lhsT: plane0 = ZBIG*I, plane1 = src-maker
                off = li * 2 * 128
                return bass.AP(tensor=blob8.tensor,
                               offset=blob8.offset + off,
                               ap=[blob8.ap[0], [128, 2], [1, 128]])

            def ladjz_rhs(g, t):
                # [128, 2, N] DR rhs: plane0 = A01 chunk t, plane1 = per-map
                lz = g_in[g][2]
                return bass.AP(tensor=lz.tensor,
                               offset=lz.offset + t * N,
                               ap=[lz.ap[0], [NT * N, 2], [1, N]])

            def leaky_act(out, in_, alpha, bias=None, accum_out=None):
                if use_prelu:
                    kw = {}
                    if bias is not None:
                        kw["bias"] = bias
                    if accum_out is not None:
                        kw["accum_out"] = accum_out
                    nc.scalar.activation(out=out, in_=in_, func=AF.Prelu,
                                         alpha=alpha, **kw)
                    return
                src = in_
                if bias is not None:
                    t = wpool.tile(list(out.shape), F32, tag="lk_t", bufs=4,
                                   name="lkb")
                    nc.scalar.activation(out=t, in_=in_, func=AF.Identity,
                                         bias=bias)
                    src = t
                nc.vector.scalar_tensor_tensor(
                    out=out, in0=src, scalar=alpha, in1=src,
                    op0=AT.mult, op1=AT.max)
                if accum_out is not None:
                    scr = wpool.tile(list(out.shape), F32, tag="lk_t2",
                                     bufs=4, name="lks")
                    nc.vector.tensor_scalar(out=scr, in0=out, scalar1=1.0,
                                            scalar2=0.0, op0=AT.mult,
                                            op1=AT.add, accum_out=accum_out)

            def dump_t(nm, src_ap, shape):
                if not dump:
                    return
                t = wpool.tile(shape, F32, tag="dumpbuf", bufs=2,
                               name="dump" + nm)
                nc.vector.tensor_copy(t, src_ap)
                dst = d_dbg[nm]
                sl = tuple([slice(None)] * len(shape))
                nc.sync.dma_start(out=dst[sl], in_=t)

            st = [dict() for _ in range(G)]

            # ================== phase 1: embeddings ==================
            for g in range(G):
                arow, prow, ladj8, amrow, pmrow = g_in[g]
                # atom one-hot -> avT [CD, N] (broadcast via PE)
                ab_ps = pssq.tile([128, N], F32, tag="sq", name="ab_ps")
                nc.tensor.matmul(ab_ps, onesrow, arow, start=True, stop=True)
                ohA = wpool.tile([128, N], MD, tag="t1k", bufs=6, name="ohA")
                nc.vector.tensor_tensor(out=ohA, in0=ab_ps, in1=iof[:, :N],
                                        op=AT.is_equal)
                avT_ps = pssq.tile([128, N], F32, tag="sq", name="avT_ps")
                nc.tensor.matmul(avT_ps, Eat, ohA, start=True, stop=True)
                avT = wpool.tile([128, N], MD, tag=f"avT{g}", bufs=1,
                                 name="avT")
                nc.scalar.copy(avT, avT_ps)
                st[g]["avT"] = avT

                # amino one-hot -> padded pv
                # dual-plane fp8 pv: plane1[k] = plane0[k+1] so DoubleRow
                # i-pairs read k-tiles at a 16B-aligned plane stride
                if CONV_FP8:
                    pv = apool.tile([PD, 2, PV8W], F8E4, tag=f"pv{g}_0",
                                    bufs=1, name="pv")
                    nc.vector.memset(pv[:, 0, :PADL], 0.0)
                    nc.vector.memset(pv[:, 0, PADL + L :], 0.0)
                    nc.vector.memset(pv[:, 1, : PADL - 1], 0.0)
                    nc.vector.memset(pv[:, 1, PADL - 1 + L :], 0.0)
                else:
                    pv = apool.tile([PD, PVW], MD, tag=f"pv{g}_0", bufs=1,
                                    name="pv")
                    nc.vector.memset(pv[:, :PADL], 0.0)
                    nc.vector.memset(pv[:, PADL + L :], 0.0)
                for c in range(2):
                    pb_ps = pssq.tile([128, N], F32, tag="sq", name="pb_ps")
                    nc.tensor.matmul(pb_ps, onesrow,
                                     prow[:, c * 512 : (c + 1) * 512],
                                     start=True, stop=True)
                    ohP = wpool.tile([NAM, N], MD, tag="t1k", bufs=6,
                                     name="ohP")
                    nc.vector.tensor_tensor(
                        out=ohP, in0=pb_ps[:NAM, :],
                        in1=iof[:NAM, c * 512 : (c + 1) * 512],
                        op=AT.is_equal)
                    pvT_ps = pscv.tile([PD, 512], F32, tag="cv",
                                       name="pvT_ps")
                    nc.tensor.matmul(pvT_ps, Eam, ohP, start=True, stop=True)
                    if CONV_FP8:
                        nc.scalar.copy(
                            pv[:, 0, PADL + c * 512 : PADL + (c + 1) * 512],
                            pvT_ps)
                        nc.vector.tensor_copy(
                            pv[:, 1, PADL - 1 + c * 512 :
                               PADL - 1 + (c + 1) * 512],
                            pv[:, 0, PADL + c * 512 : PADL + (c + 1) * 512])
                    else:
                        nc.scalar.copy(
                            pv[:, PADL + c * 512 : PADL + (c + 1) * 512],
                            pvT_ps)
                st[g]["pv"] = pv

            # conv machinery + L1 early (warms the PE during phase 2)
            # ================== conv machinery ==================
            conv_steps = []

            def make_conv_layer(lyr):
                pvo_l, cv = [], {}
                last = lyr == LC - 1
                for g in range(G):
                    if last:
                        pvo = apool.tile([PD, PVW], MD, tag=f"pvf{g}",
                                         bufs=1, name="pvo")
                    elif CONV_FP8:
                        pvo = apool.tile([PD, 2, PV8W], F8E4,
                                         tag=f"pv{g}_{1 - lyr % 2}", bufs=1,
                                         name="pvo")
                    else:
                        pvo = apool.tile([PD, PVW], MD,
                                         tag=f"pv{g}_{1 - lyr % 2}", bufs=1,
                                         name="pvo")
                    pvo_l.append(pvo)
                steps = []

                def mk_group(g, c):
                    def run():
                        if c == 0:
                            if last or not CONV_FP8:
                                nc.vector.memset(pvo_l[g][:, :PADL], 0.0)
                                nc.vector.memset(pvo_l[g][:, PADL + L :],
                                                 0.0)
                            else:
                                nc.vector.memset(pvo_l[g][:, 0, :PADL], 0.0)
                                nc.vector.memset(
                                    pvo_l[g][:, 0, PADL + L :], 0.0)
                                nc.vector.memset(
                                    pvo_l[g][:, 1, : PADL - 1], 0.0)
                                nc.vector.memset(
                                    pvo_l[g][:, 1, PADL - 1 + L :], 0.0)
                        cv[g] = pscv.tile([PD, 512], F32, tag="cv",
                                          name=f"cv{g}")
                        pv = st[g]["pv"]
                        if CONV_FP8:
                            for pr in range(NPR):
                                nc.tensor.matmul(
                                    cv[g], MiT8_v(lyr, pr),
                                    bass.AP(tensor=pv.tensor,
                                            offset=pv.offset + c * 512
                                            + 2 * pr,
                                            ap=[pv.ap[0], [PV8W, 2],
                                                [1, 512]]),
                                    start=(pr == 0), stop=(pr == NPR - 1),
                                    perf_mode=DR)
                        else:
                            for i in range(KW):
                                nc.tensor.matmul(
                                    cv[g], MiT_v(lyr, i),
                                    pv[:, c * 512 + i : c * 512 + i + 512],
                                    start=(i == 0), stop=(i == KW - 1))
                    return run

                def mk_relu(g, c):
                    def run():
                        if last or not CONV_FP8:
                            nc.vector.tensor_scalar(
                                out=pvo_l[g][:, PADL + c * 512 :
                                             PADL + (c + 1) * 512],
                                in0=cv[g], scalar1=cb[:, lyr : lyr + 1],
                                scalar2=0.0, op0=AT.add, op1=AT.max)
                        else:
                            nc.vector.tensor_scalar(
                                out=pvo_l[g][:, 0, PADL + c * 512 :
                                             PADL + (c + 1) * 512],
                                in0=cv[g], scalar1=cb[:, lyr : lyr + 1],
                                scalar2=0.0, op0=AT.add, op1=AT.max)
                            nc.vector.tensor_copy(
                                pvo_l[g][:, 1, PADL - 1 + c * 512 :
                                         PADL - 1 + (c + 1) * 512],
                                pvo_l[g][:, 0, PADL + c * 512 :
                                         PADL + (c + 1) * 512])
                    return run

                for c in range(2):
                    for g in range(G):
                        steps.append(mk_group(g, c))
                        steps.append(mk_relu(g, c))

                def finish():
                    for g in range(G):
                        st[g]["pv"] = pvo_l[g]
                    if dump and lyr == LC - 1:
                        dump_t("pv3", pvo_l[0], [128, PVW])
                steps.append(finish)
                return steps

            for lyr in range(LC):
                conv_steps.extend(make_conv_layer(lyr))

            # prot head rides the conv queue: each chunk is ready as soon
            # as the final conv relu for it has run
            opacc = {}

            def mk_prot(g, c):
                def run():
                    pv = st[g]["pv"]
                    pvt = psz.tile([128, 2, N], F32, tag="z", name="pv_ps")
                    pv_ps = pvt[:LAT, 0, :]
                    nc.tensor.matmul(pv_ps, Wa,
                                     pv[:, PADL + c * 512 :
                                        PADL + (c + 1) * 512],
                                     start=True, stop=True)
                    p_v = wpool.tile([LAT, 512], MD, tag="p_v", bufs=4,
                                     name="p_v")
                    pacc = rpool.tile([LAT, 1], F32, tag="c1", bufs=8,
                                      name="pacc")
                    leaky_act(p_v, pv_ps, ALPHA, bias=ba, accum_out=pacc)
                    opacc.setdefault(g, []).append(pacc)
                return run

            for c in range(2):
                for g in range(G):
                    conv_steps.append(mk_prot(g, c))

            def pop_conv(k):
                for _ in range(k):
                    if conv_steps:
                        conv_steps.pop(0)()


            # ================== phase 2: per-graph prep ==================
            for g in range(G):
                avT = st[g]["avT"]
                # all heads' Wh -> whsb_all[p, t, h, 0:64]; col 64 = 1.0
                whsb_all = wpool.tile([128, NT, H, GD + 2], MD,
                                      tag=f"whsb{g}", bufs=1, name="whsb_all")
                nc.vector.memset(whsb_all[:, :, :, GD : GD + 1], 1.0)
                for half in range(2):
                    wh_all = pssq.tile([128, 2, H * GD], F32, tag="sq",
                                       name="wh_all")
                    for t2 in range(2):
                        t = half * 2 + t2
                        nc.tensor.matmul(
                            wh_all[:, t2, :],
                            avT[:, t * 128 : (t + 1) * 128],
                            Wg_flat, start=True, stop=True)
                    nc.scalar.copy(
                        out=bass.AP(
                            tensor=whsb_all.tensor,
                            offset=whsb_all.offset
                            + half * 2 * H * (GD + 2),
                            ap=[whsb_all.ap[0], [H * (GD + 2), 2],
                                [GD + 2, H], [1, GD]]),
                        in_=wh_all)
                st[g]["whsb"] = whsb_all

                # dst bias columns for all heads: dstT[j, t*H+h] =
                # a2_h.Wh_j - ZBIG (the -ZBIG cancels plane0's ZBIG*A01)
                dst_ps = pssq.tile([128, NT * H], F32, tag="sq",
                                   name="dst_ps")
                for t in range(NT):
                    nc.tensor.matmul(dst_ps[:, t * H : (t + 1) * H],
                                     avT[:, t * 128 : (t + 1) * 128], Wa2h,
                                     start=True, stop=True)
                dstT = rpool.tile([128, NT * H], F32, tag=f"dstT{g}", bufs=1,
                                  name="dstT")
                nc.vector.tensor_scalar(out=dstT, in0=dst_ps, scalar1=-ZBIG,
                                        scalar2=None, op0=AT.add)
                st[g]["dstT"] = dstT
                # z rhs plane1 for head maps: avT in fp8e4
                ladjz = g_in[g][2]
                for t in range(NT):
                    nc.vector.tensor_copy(ladjz[:, 1, t, :], avT)


            # ============ attention z/U (per head or out-layer) ============
            def att_zU(g, li, bias4, dump_zu=False):
                """One DR matmul per chunk: z = ZBIG*A01 + src; dst - ZBIG
                enters as the Prelu bias. Returns U [128, NT, N] fp16."""
                ee = apool.tile([128, NT, N], MD, tag="ee", bufs=2, name="ee")
                for th in range(2):
                    zps = psz.tile([128, 2, N], F32, tag="z", name="zps")
                    for t2 in range(2):
                        t = th * 2 + t2
                        nc.tensor.matmul(zps[:, t2, :], zlhs_v(li),
                                         ladjz_rhs(g, t), start=True,
                                         stop=True, perf_mode=DR)
                    for t2 in range(2):
                        t = th * 2 + t2
                        if use_prelu:
                            nc.scalar.activation(out=ee[:, t, :],
                                                 in_=zps[:, t2, :],
                                                 func=AF.Prelu, alpha=ALPHA,
                                                 bias=bias4[t])
                        else:
                            eesc = wpool.tile([128, N], F32, tag="eesc",
                                              bufs=4, name="eesc")
                            nc.scalar.activation(out=eesc, in_=zps[:, t2, :],
                                                 func=AF.Identity,
                                                 bias=bias4[t])
                            nc.vector.scalar_tensor_tensor(
                                out=ee[:, t, :], in0=eesc, scalar=ALPHA,
                                in1=eesc, op0=AT.mult, op1=AT.max)
                if dump_zu:
                    dump_t("zm0", ee, [128, NT, N])
                U = apool.tile([128, NT, N], MD, tag="U", bufs=2, name="U")
                nc.scalar.activation(out=U, in_=ee, func=AF.Exp)
                if dump_zu:
                    dump_t("U0", U, [128, NT, N])
                return U

            def elu_norm_pair(hp0, hp1, dst):
                """Normalize two heads' hp [65, N] (row 64 = rowsum) and
                elu into packed dst [128, N]."""
                zrw2 = rpool.tile([33, N], F32, tag="zrw2", bufs=3,
                                  name="zrw2")
                nc.gpsimd.memset(zrw2, 1.0)
                nc.vector.tensor_copy(zrw2[0:1, :], hp0[64:65, :])
                nc.vector.tensor_copy(zrw2[32:33, :], hp1[64:65, :])
                rinv2 = rpool.tile([33, N], F32, tag="zrw2", bufs=3,
                                   name="rinv2")
                nc.vector.reciprocal_approx_fast(out=rinv2, in_=zrw2)
                rinv16 = rpool.tile([33, N], MD, tag="ri16", bufs=2,
                                    name="rinv16")
                nc.vector.tensor_scalar(out=rinv16, in0=rinv2, scalar1=1.0,
                                        scalar2=None, op0=AT.mult)
                rb_ps = pssq.tile([128, N], F32, tag="sq", name="rb_ps")
                nc.tensor.matmul(rb_ps, E2, rinv16, start=True, stop=True)
                rb = wpool.tile([128, N], MD, tag="rb", bufs=2, name="rb")
                nc.vector.tensor_copy(rb, rb_ps)
                hpn = wpool.tile([128, N], MD, tag="elu", bufs=4, name="hpn")
                nc.vector.scalar_tensor_tensor(
                    out=hpn[0:64, :], in0=hp0[:64, :], scalar=1.0,
                    in1=rb[0:64, :], op0=AT.mult, op1=AT.mult)
                nc.vector.scalar_tensor_tensor(
                    out=hpn[64:128, :], in0=hp1[:64, :], scalar=1.0,
                    in1=rb[64:128, :], op0=AT.mult, op1=AT.mult)
                xm = wpool.tile([128, N], MD, tag="elu", bufs=4, name="xm")
                nc.vector.tensor_scalar(out=xm, in0=hpn, scalar1=0.0,
                                        scalar2=None, op0=AT.min)
                em = wpool.tile([128, N], MD, tag="elu", bufs=4, name="em")
                nc.scalar.activation(out=em, in_=xm, func=AF.Exp)
                nc.vector.scalar_tensor_tensor(
                    out=dst, in0=em, scalar=-1.0, in1=hpn,
                    op0=AT.add, op1=AT.max)

            # ================== phase 3: head maps + conv ==================
            for g in range(G):
                m01 = wpool.tile([128, N], MD, tag=f"m01{g}", bufs=1,
                                 name="m01")
                m23 = wpool.tile([128, N], MD, tag=f"m23{g}", bufs=1,
                                 name="m23")
                st[g]["multi"] = (m01, m23)
                st[g]["hp"] = {}

            # software-pipelined attention: emit z/U of map k+1 before the
            # hp matmuls of map k so the PE never stalls on the exp chain
            pend = None          # (g, h, U) waiting for its hp emission
            pair_hps = {}

            def emit_hp(g, h, U):
                whsb_all = st[g]["whsb"]
                hp_t = pshp.tile([128, N], F32, tag="hp", name="hp_ps")
                hp = hp_t[0:65, :]
                for t in range(NT):
                    nc.tensor.matmul(hp, whsb_all[:, t, h, : GD + 1],
                                     U[:, t, :],
                                     start=(t == 0), stop=(t == NT - 1))
                pair_hps.setdefault(g, []).append(hp)
                if len(pair_hps[g]) == 2:
                    hps = pair_hps.pop(g)
                    elu_norm_pair(hps[0], hps[1], st[g]["multi"][h // 2])

            pop_conv(9)
            for hp_i in range(H // 2):
                for g in range(G):
                    dstT = st[g]["dstT"]
                    for h2 in range(2):
                        h = hp_i * 2 + h2
                        U = att_zU(g, h,
                                   [dstT[:, t * H + h : t * H + h + 1]
                                    for t in range(NT)],
                                   dump_zu=(dump and g == 0 and h == 0))
                        if hp_i == 0 and g == 0 and h2 == 0:
                            late_dmas()
                        if pend is not None:
                            emit_hp(*pend)
                        pend = (g, h, U)
                        pop_conv(1)
            if pend is not None:
                emit_hp(*pend)
                pend = None

            if dump:
                dump_t("m01", st[0]["multi"][0], [128, N])
                dump_t("m23", st[0]["multi"][1], [128, N])

            # ================== phase 4: GAT output layer ==================
            for g in range(G):
                m01, m23 = st[g]["multi"]
                multi = [m01, m23]
                wh2_ps = pssq.tile([128, NT, CD], F32, tag="sq",
                                   name="wh2_ps")
                for t in range(NT):
                    for c in range(2):
                        nc.tensor.matmul(
                            wh2_ps[:, t, :],
                            multi[c][:, t * 128 : (t + 1) * 128],
                            Wgo_v(c), start=(c == 0), stop=(c == 1))
                wh2sb = wpool.tile([128, NT, CD], MD, tag=f"wh2{g}", bufs=1,
                                   name="wh2sb")
                nc.vector.tensor_copy(wh2sb, wh2_ps)
                st[g]["wh2sb"] = wh2sb

                # out-layer z rhs plane1: q[k,i] = a1go[k,0]*m01 + a1go[k,1]
                # *m23 (ones-lhsT row-sums it into src2)
                qt1 = wpool.tile([128, N], MD, tag="qt", bufs=4, name="qt1")
                nc.vector.tensor_scalar(out=qt1, in0=m01, scalar1=wa1c0,
                                        scalar2=None, op0=AT.mult)
                qt2 = wpool.tile([128, N], MD, tag="qt", bufs=4, name="qt2")
                nc.vector.tensor_scalar(out=qt2, in0=m23, scalar1=wa1c1,
                                        scalar2=None, op0=AT.mult)
                ladjz = g_in[g][2]
                for t in range(NT):
                    nc.vector.tensor_tensor(out=ladjz[:, 1, t, :], in0=qt1,
                                            in1=qt2, op=AT.add)
                # out-layer dst bias columns: dstT2[j, t] = a2go.xo_j - ZBIG
                d2_ps = pssq.tile([128, NT], F32, tag="sq", name="d2_ps")
                for t in range(NT):
                    for c in range(2):
                        nc.tensor.matmul(
                            d2_ps[:, t : t + 1],
                            multi[c][:, t * 128 : (t + 1) * 128],
                            Wgoa2[:, c : c + 1],
                            start=(c == 0), stop=(c == 1))
                dstT2 = rpool.tile([128, NT], F32, tag=f"d2T{g}", bufs=1,
                                   name="dstT2")
                nc.vector.tensor_scalar(out=dstT2, in0=d2_ps, scalar1=-ZBIG,
                                        scalar2=None, op0=AT.add)
                st[g]["dstT2"] = dstT2
                pop_conv(3)

            oU, ohp, ors, orinv, orb = {}, {}, {}, {}, {}
            for g in range(G):
                dstT2 = st[g]["dstT2"]
                oU[g] = att_zU(g, H,
                               [dstT2[:, t : t + 1] for t in range(NT)])
                pop_conv(1)
            for g in range(G):
                wh2sb = st[g]["wh2sb"]
                hp2 = pshp.tile([128, N], F32, tag="hp", name="hp2_ps")
                for t in range(NT):
                    nc.tensor.matmul(hp2, wh2sb[:, t, :], oU[g][:, t, :],
                                     start=(t == 0), stop=(t == NT - 1))
                rs_t = psz.tile([128, 2, N], F32, tag="z", name="rs_ps")
                rs_ps = rs_t[:, 0, :]
                for t in range(NT):
                    nc.tensor.matmul(rs_ps[0:1, :], ones16c[:, 0:1],
                                     oU[g][:, t, :],
                                     start=(t == 0), stop=(t == NT - 1))
                ohp[g] = hp2
                ors[g] = rs_ps
                pop_conv(1)
            for g in range(G):
                rinv = rpool.tile([1, N], F32, tag="zrwo", bufs=6,
                                  name="rinv")
                nc.vector.reciprocal_approx_fast(out=rinv,
                                                 in_=ors[g][0:1, :])
                if dump and g == 0:
                    dump_t("rinv0", rinv, [1, N])
                rinv16 = rpool.tile([1, N], MD, tag="ri16o", bufs=2,
                                    name="rinv16o")
                nc.vector.tensor_scalar(out=rinv16, in0=rinv, scalar1=1.0,
                                        scalar2=None, op0=AT.mult)
                orinv[g] = rinv16
            for g in range(G):
                rb_ps = pssq.tile([128, N], F32, tag="sq", name="rb2_ps")
                nc.tensor.matmul(rb_ps, onesrow, orinv[g], start=True,
                                 stop=True)
                rb = wpool.tile([128, N], MD, tag="rb", bufs=2, name="rbo")
                nc.vector.tensor_copy(rb, rb_ps)
                orb[g] = rb
            for g in range(G):
                xT = wpool.tile([CD, N], MD, tag=f"xT{g}", bufs=1, name="xT")
                hpn = wpool.tile([128, N], MD, tag="elu", bufs=4,
                                 name="hpno")
                nc.vector.scalar_tensor_tensor(
                    out=hpn, in0=ohp[g], scalar=1.0, in1=orb[g],
                    op0=AT.mult, op1=AT.mult)
                xm = wpool.tile([128, N], MD, tag="elu", bufs=4, name="xmo")
                nc.vector.tensor_scalar(out=xm, in0=hpn, scalar1=0.0,
                                        scalar2=None, op0=AT.min)
                em = wpool.tile([128, N], MD, tag="elu", bufs=4, name="emo")
                nc.scalar.activation(out=em, in_=xm, func=AF.Exp)
                nc.vector.scalar_tensor_tensor(
                    out=xT, in0=em, scalar=-1.0, in1=hpn,
                    op0=AT.add, op1=AT.max)
                st[g]["xT"] = xT
                if dump and g == 0:
                    dump_t("xT", xT, [128, N])
                pop_conv(2)

            # ============ phase 5: comp head + conv (graph-stepped) ========
            oav, oavec, oav2 = {}, {}, {}
            for g in range(G):
                av_t = pshp.tile([128, N], F32, tag="hp", name="av_ps")
                oav[g] = av_t[:LAT, :]
                nc.tensor.matmul(oav[g], Wc, st[g]["xT"], start=True,
                                 stop=True)
                pop_conv(1)
            for g in range(G):
                avec = wpool.tile([LAT, N], MD, tag="avec", bufs=2,
                                  name="avec")
                leaky_act(avec, oav[g], ALPHA, bias=bc)
                oavec[g] = avec
            for g in range(G):
                av2_t = pshp.tile([128, N], F32, tag="hp", name="av2_ps")
                oav2[g] = av2_t[:LAT, :]
                nc.tensor.matmul(oav2[g], Wa, oavec[g], start=True,
                                 stop=True)
                pop_conv(1)
            for g in range(G):
                a_v = wpool.tile([LAT, N], MD, tag="avec", bufs=2,
                                 name="a_v")
                comp_acc = rpool.tile([LAT, 1], F32, tag="c1", bufs=8,
                                      name="comp_acc")
                leaky_act(a_v, oav2[g], ALPHA, bias=ba, accum_out=comp_acc)
                st[g]["comp_acc"] = comp_acc
            for g in range(G):
                comp_acc = st[g]["comp_acc"]
                amrow = g_in[g][3]
                amscr = rpool.tile([1, N], F32, tag="r2k", bufs=6,
                                   name="amscr")
                amsum = rpool.tile([1, 1], F32, tag="c2", bufs=12,
                                   name="amsum")
                nc.vector.tensor_scalar(out=amscr, in0=amrow, scalar1=1.0,
                                        scalar2=0.0, op0=AT.mult, op1=AT.add,
                                        accum_out=amsum)
                amsb = rpool.tile([128, 1], F32, tag="c2", bufs=12,
                                  name="amsb")
                nc.gpsimd.partition_broadcast(amsb, amsum)
                amr = rpool.tile([128, 1], F32, tag="c2", bufs=12,
                                 name="amr")
                nc.vector.reciprocal(amr, amsb)
                cp = rpool.tile([128, 2], F32, tag="cp", bufs=4, name="cp")
                nc.vector.tensor_scalar(out=cp[:, 0:1], in0=comp_acc,
                                        scalar1=amr, scalar2=None,
                                        op0=AT.mult)
                st[g]["cp"] = cp
                if dump and g == 0:
                    dump_t("comp", comp_acc, [LAT, 1])
                pop_conv(3)

            pop_conv(len(conv_steps))

            # ========= phase 6: prot pooling + prediction (prot head ran
            # via the conv queue) =====
            for g in range(G):
                cp = st[g]["cp"]
                prot_acc = rpool.tile([LAT, 1], F32, tag="c1", bufs=8,
                                      name="prot_acc")
                nc.vector.tensor_tensor(out=prot_acc, in0=opacc[g][0],
                                        in1=opacc[g][1], op=AT.add)
                if dump and g == 0:
                    dump_t("prot", prot_acc, [LAT, 1])

                pmrow = g_in[g][4]
                pmscr = rpool.tile([1, L], F32, tag="r4k", bufs=2,
                                   name="pmscr")
                pmsum = rpool.tile([1, 1], F32, tag="c2", bufs=12,
                                   name="pmsum")
                nc.vector.tensor_scalar(out=pmscr, in0=pmrow, scalar1=1.0,
                                        scalar2=0.0, op0=AT.mult, op1=AT.add,
                                        accum_out=pmsum)
                pmsb = rpool.tile([128, 1], F32, tag="c2", bufs=12,
                                  name="pmsb")
                nc.gpsimd.partition_broadcast(pmsb, pmsum)
                pmr = rpool.tile([128, 1], F32, tag="c2", bufs=12, name="pmr")
                nc.vector.reciprocal(pmr, pmsb)
                nc.vector.tensor_scalar(out=cp[:, 1:2], in0=prot_acc,
                                        scalar1=pmr, scalar2=None,
                                        op0=AT.mult)

                lr2 = rpool.tile([128, 2], F32, tag="cp", bufs=4, name="lr2")
                leaky_act(lr2, cp, ALPHA * ALPHA)
                dscr = rpool.tile([128, 2], F32, tag="cp", bufs=4,
                                  name="dscr")
                dacc = rpool.tile([128, 1], F32, tag="c1", bufs=8,
                                  name="dacc")
                nc.vector.scalar_tensor_tensor(
                    out=dscr, in0=lr2, scalar=1.0, in1=pw,
                    op0=AT.mult, op1=AT.mult, accum_out=dacc)
                fin_ps = pssq.tile([128, N], F32, tag="sq", name="fin_ps")
                nc.tensor.matmul(fin_ps[0:1, 0:1], dacc, ones_col,
                                 start=True, stop=True)
                res = rpool.tile([1, 1], F32, tag="c2", bufs=12, name="res")
                nc.scalar.activation(out=res, in_=fin_ps[0:1, 0:1],
                                     func=AF.Identity, bias=pb)
                nc.sync.dma_start(out=d_out[g : g + 1, :], in_=res)

    return nc


def preprocess(inputs, mm_bf16=False):
    """Host-side prep: shard over cores, transpose/reshape weights."""
    import ml_dtypes
    md = np.float16
    f8 = ml_dtypes.float8_e4m3fn
    atoms = np.asarray(inputs["atoms"]).astype(np.float32)
    atoms_mask = np.asarray(inputs["atoms_mask"]).astype(np.float32)
    adjacency = np.asarray(inputs["adjacency"])
    amino = np.asarray(inputs["amino"]).astype(np.float32)
    amino_mask = np.asarray(inputs["amino_mask"]).astype(np.float32)
    E_atom = np.asarray(inputs["E_atom"]).astype(np.float32)
    E_amino = np.asarray(inputs["E_amino"]).astype(np.float32)
    W_gat = np.asarray(inputs["W_gat"]).astype(np.float32)
    a_gat = np.asarray(inputs["a_gat"]).astype(np.float32)
    W_go = np.asarray(inputs["W_go"]).astype(np.float32)
    a_go = np.asarray(inputs["a_go"]).astype(np.float32)
    W_comp_w = np.asarray(inputs["W_comp_w"]).astype(np.float32)
    W_comp_b = np.asarray(inputs["W_comp_b"]).astype(np.float32)
    conv_w = np.asarray(inputs["conv_w"]).astype(np.float32)
    conv_b = np.asarray(inputs["conv_b"]).astype(np.float32)
    W_att_w = np.asarray(inputs["W_att_w"]).astype(np.float32)
    W_att_b = np.asarray(inputs["W_att_b"]).astype(np.float32)
    pred_w = np.asarray(inputs["pred_w"]).astype(np.float32)
    pred_b = np.asarray(inputs["pred_b"]).astype(np.float32)

    ladjT = (adjacency.transpose(0, 2, 1) > 0).astype(np.float32)
    ladjT_r = np.ascontiguousarray(
        ladjT.reshape(B, NT, 128, N).transpose(0, 2, 1, 3))

    E_atom_pad = np.zeros((128, CD), np.float32)
    E_atom_pad[:NA] = E_atom

    MiT = np.zeros((LC, KW, PD, PD), np.float32)
    din = np.arange(PD)[:, None]
    dout = np.arange(PD)[None, :]
    v = din - dout + (KW // 2)
    valid = (v >= 0) & (v < KW)
    vc = np.clip(v, 0, KW - 1)
    for lyr in range(LC):
        for i in range(KW):
            MiT[lyr, i] = np.where(valid, conv_w[lyr, 0, 0, i, vc], 0.0)
    MiT_r = np.ascontiguousarray(MiT.transpose(2, 0, 1, 3))
    NPRl = (KW + 1) // 2
    MiT8 = np.zeros((LC, NPRl, 2, PD, PD), np.float32)
    for lyr in range(LC):
        for pr in range(NPRl):
            MiT8[lyr, pr, 0] = MiT[lyr, 2 * pr]
            if 2 * pr + 1 < KW:
                MiT8[lyr, pr, 1] = MiT[lyr, 2 * pr + 1]
    MiT8_r = np.ascontiguousarray(MiT8.transpose(3, 0, 1, 2, 4))

    W_gat_r = np.ascontiguousarray(W_gat.transpose(1, 0, 2))
    Wa1h = np.einsum("hpq,hq->ph", W_gat, a_gat[:, :GD])
    Wa2h = np.einsum("hpq,hq->ph", W_gat, a_gat[:, GD:])

    W_go_r = np.ascontiguousarray(
        W_go.reshape(2, 128, CD).transpose(1, 0, 2))
    Wgo_a2 = (W_go @ a_go[CD:]).reshape(2, 128).T     # (128, 2)
    Wgo_a1 = (W_go @ a_go[:CD]).reshape(2, 128).T

    E2 = np.zeros((33, 128), np.float32)
    E2[0, 0:64] = 1.0
    E2[32, 64:128] = 1.0

    # pack fp16 weight blob (offsets mirror kernel W16O)
    W16N = 1484 if CONV_FP8 else 1484 + LC * KW * PD
    blob16 = np.zeros((128, W16N), np.float32)

    def put16(off, arr):
        a = np.asarray(arr, np.float32)
        blob16[: a.shape[0], off : off + a.shape[1]] = a

    put16(0, E_atom_pad)
    put16(128, E_amino)
    put16(256, W_gat_r.reshape(CD, H * GD))
    put16(512, Wa2h)
    put16(708, W_go_r.reshape(128, 2 * CD))
    put16(964, Wgo_a2)
    put16(1100, E2)
    put16(1228, W_comp_w.T)
    put16(1356, W_att_w.T)
    if not CONV_FP8:
        put16(1484, MiT_r.reshape(PD, LC * KW * PD))

    blobf = np.zeros((128, 7), np.float32)
    blobf[:, 0:2] = Wgo_a1
    blobf[:LAT, 2] = W_comp_b
    blobf[:LAT, 3] = W_att_b
    blobf[:LAT, 4] = pred_w[0, :LAT]
    blobf[:LAT, 5] = pred_w[0, LAT:]
    blobf[0, 6] = pred_b[0]

    # fp8 blob: z lhsT planes (ZBIG*I | src-maker) per map, then MiT8
    B8Wl = NZL * 2 * 128 + (LC * NPRl * 2 * PD if CONV_FP8 else 0)
    blob8 = np.zeros((128, B8Wl), np.float32)
    zlarr = np.zeros((128, NZL, 2, 128), np.float32)
    for li in range(NZL):
        zlarr[:, li, 0, :] = ZBIG * np.eye(128, dtype=np.float32)
    for h in range(H):
        zlarr[:, h, 1, :] = Wa1h[:, h : h + 1]
    zlarr[:, H, 1, :] = 1.0
    blob8[:, : NZL * 2 * 128] = zlarr.reshape(128, NZL * 2 * 128)
    if CONV_FP8:
        blob8[:, NZL * 2 * 128 :] = MiT8_r.reshape(PD, LC * NPRl * 2 * PD)

    shared = {
        "blob16": blob16.astype(md),
        "blobf": blobf,
        "blob8": blob8.astype(ml_dtypes.float8_e4m3fn),
        "conv_b": np.ascontiguousarray(conv_b.reshape(LC, 1)),
    }
    in_maps = []
    for c in range(NCORES):
        sl = slice(c * G, (c + 1) * G)
        m = dict(shared)
        m["atoms_f"] = np.ascontiguousarray(atoms[sl]).astype(md)
        m["atoms_mask"] = np.ascontiguousarray(atoms_mask[sl])
        m["ladj8"] = np.ascontiguousarray(ladjT_r[sl]).astype(f8)
        m["amino_f"] = np.ascontiguousarray(amino[sl]).astype(md)
        m["amino_mask"] = np.ascontiguousarray(amino_mask[sl])
        in_maps.append(m)
    return in_maps


_CACHED_NC = None


def kernel(**inputs) -> np.ndarray:
    global _CACHED_NC
    from concourse.bass_utils import run_bass_kernel_spmd

    if _CACHED_NC is None:
        nc = build_core_program()
        nc.finalize()
        _CACHED_NC = nc
    nc = _CACHED_NC
    in_maps = preprocess(inputs)
    res = run_bass_kernel_spmd(nc, in_maps, core_ids=list(range(NCORES)))
    out = np.concatenate([res.results[c]["out"] for c in range(NCORES)], axis=0)
    return out.astype(np.float32)

